# revision 18
# baseline (speedup 1.0000x reference)
import sys

sys.path.insert(0, "/opt/trn_rl_repo")

import numpy as np

D_MODEL = 1024
NUM_HEADS = 16
HEAD_DIM = 64
B = 2
S = 2048
N_CORES = 8
HG = 4          # head-groups (cores per batch)
HPC = 4         # heads per core
DL = 256        # local feature width per core (HPC * HEAD_DIM)

_cache = {}
last_exec_time_ns = None


def _build(has_qkvb):
    import concourse.bacc as bacc
    import concourse.mybir as mybir
    import concourse.tile as tile

    F32 = mybir.dt.float32
    BF16 = mybir.dt.bfloat16
    Exp = mybir.ActivationFunctionType.Exp
    mult = mybir.AluOpType.mult
    is_ge = mybir.AluOpType.is_ge

    nc = bacc.Bacc("TRN2", target_bir_lowering=False, debug=False)
    xT_d = nc.dram_tensor("xT", (D_MODEL, S), BF16, kind="ExternalInput")
    wq_d = nc.dram_tensor("wqkvT", (D_MODEL, 3 * DL), BF16, kind="ExternalInput")
    wo_d = nc.dram_tensor("woT", (DL, D_MODEL), BF16, kind="ExternalInput")
    if has_qkvb:
        qb_d = nc.dram_tensor("qb", (1, 3 * DL), BF16, kind="ExternalInput")
    out_d = nc.dram_tensor("out", (S, D_MODEL), F32, kind="ExternalOutput")

    with tile.TileContext(nc) as tc:
        with tc.tile_pool(name="persist", bufs=1) as persist:
            xt = [persist.tile([128, S], BF16, name=f"xt{i}") for i in range(8)]
            wq = [persist.tile([128, 3 * DL], BF16, name=f"wq{i}") for i in range(8)]
            # Q/K packed per head-pair p: partitions 0:64 head 2p, 64:128 head 2p+1
            QT = [persist.tile([128, S], BF16, name=f"QT{p}") for p in range(2)]
            KT = [persist.tile([128, S], BF16, name=f"KT{p}") for p in range(2)]
            # V augmented, single tile: [pair, st, head-parity, (v|ones), 64]
            VA = persist.tile([128, 2, 16, 2, 2, 64], BF16, name="VA")
            # ctx pair-packed: head 2p at partitions 0:64, head 2p+1 at 64:128
            ctxp = [persist.tile([128, S], BF16, name=f"ctxp{p}") for p in range(2)]
            wop = [persist.tile([128, D_MODEL], BF16, name=f"wop{p}") for p in range(2)]

            # input DMAs spread across engine queues for issue parallelism
            # first 512 cols of x land first so the n=0 projection group is
            # not serialized behind the full x transfer; spread across the
            # three DMA-capable queues (sync, gpsimd, scalar)
            qeng = [nc.sync, nc.sync, nc.sync, nc.gpsimd,
                    nc.gpsimd, nc.gpsimd, nc.scalar, nc.scalar]
            for i in range(8):
                qeng[i].dma_start(out=wq[i][:], in_=wq_d[128 * i:128 * (i + 1), :])
                qeng[i].dma_start(
                    out=xt[i][:, 0:512], in_=xT_d[128 * i:128 * (i + 1), 0:512])
            for p in range(2):
                nc.scalar.dma_start(out=wop[p][:], in_=wo_d[128 * p:128 * (p + 1), :])
            for i in range(8):
                qeng[i].dma_start(
                    out=xt[i][:, 512:1024], in_=xT_d[128 * i:128 * (i + 1), 512:1024])
            for i in range(4):
                nc.sync.dma_start(
                    out=xt[i][:, 1024:2048],
                    in_=xT_d[128 * i:128 * (i + 1), 1024:2048])
            for i in range(4, 8):
                nc.gpsimd.dma_start(
                    out=xt[i][:, 1024:2048],
                    in_=xT_d[128 * i:128 * (i + 1), 1024:2048])

            # ones columns of VA (v columns are overwritten by v_proj copies)
            nc.vector.memset(VA[:], 1.0)

            with tc.tile_pool(name="work", bufs=1) as work, \
                 tc.tile_pool(name="psum", bufs=1, space="PSUM") as psum:

                if has_qkvb:
                    qb_t = persist.tile([1, 3 * DL], BF16, name="qb_t")
                    nc.sync.dma_start(out=qb_t[:], in_=qb_d[:])
                    ones_t = persist.tile([1, 512], BF16, name="ones_t")
                    nc.vector.memset(ones_t[:], 1.0)

                # ACT exp-table preload during DMA wait
                warm = work.tile([1, 16], F32, name="warm")
                nc.vector.memset(warm[:], 0.0)
                nc.scalar.activation(warm[:], warm[:], Exp, scale=1.0)

                # ---- filler emitters (each yields per-matmul granularity) ----

                def qk_proj(mi, n):
                    # psq [128 qk-dims, 512 keys]; mi 0,1 = Q pairs; 2,3 = K pairs
                    dst = QT[mi] if mi < 2 else KT[mi - 2]
                    psq = psum.tile([128, 512], F32, tag="p", bufs=2, name="psq")
                    for i in range(8):
                        yield
                        nc.tensor.matmul(
                            out=psq[:],
                            lhsT=wq[i][:, 128 * mi:128 * (mi + 1)],
                            rhs=xt[i][:, 512 * n:512 * (n + 1)],
                            start=(i == 0),
                            stop=(i == 7 and not has_qkvb),
                        )
                    if has_qkvb:
                        nc.tensor.matmul(
                            out=psq[:],
                            lhsT=qb_t[0:1, 128 * mi:128 * (mi + 1)],
                            rhs=ones_t[0:1, :],
                            start=False, stop=True,
                        )
                    nc.vector.tensor_copy(out=dst[:, 512 * n:512 * (n + 1)], in_=psq[:])

                def v_proj(st):
                    # psv [128 keys, (pair, parity, 64)]
                    psv = psum.tile([128, 2, 2, 64], F32, tag="p", bufs=2, name="psv")
                    for i in range(8):
                        yield
                        nc.tensor.matmul(
                            out=psv[:],
                            lhsT=xt[i][:, 128 * st:128 * (st + 1)],
                            rhs=wq[i][:, 512:768],
                            start=(i == 0),
                            stop=(i == 7 and not has_qkvb),
                        )
                    if has_qkvb:
                        nc.tensor.matmul(
                            out=psv[:],
                            lhsT=ones_t[0:1, 0:128],
                            rhs=qb_t[0:1, 512:768],
                            start=False, stop=True,
                        )
                    nc.vector.tensor_copy(out=VA[:, :, st, :, 0, :], in_=psv[:])

                def out_proj(qm):
                    stage = work.tile([128, D_MODEL], F32, tag="st", bufs=2, name="stage")
                    for nh in range(2):
                        pso = psum.tile([128, 512], F32, tag="p", bufs=2, name="pso")
                        for p in range(2):
                            yield
                            nc.tensor.matmul(
                                out=pso[:],
                                lhsT=ctxp[p][:, 128 * qm:128 * (qm + 1)],
                                rhs=wop[p][:, 512 * nh:512 * (nh + 1)],
                                start=(p == 0), stop=(p == 1),
                            )
                        nc.vector.tensor_copy(
                            out=stage[:, 512 * nh:512 * (nh + 1)], in_=pso[:])
                    nc.sync.dma_start(out=out_d[128 * qm:128 * (qm + 1), :], in_=stage[:])

                # global attention step sequence and filler queue with gates.
                # gate = number of attention steps that must be EMITTED before
                # the filler unit may start (keeps FIFO order consistent with
                # data deps, e.g. out_proj needs its ctx block normalized).
                steps = [(p, j, m) for j in range(4) for p in range(2)
                         for m in range(4 * j + 4)]
                n_steps = len(steps)   # 80
                # step index right after block (p,j) finishes:
                end_of = {}
                acc = 0
                for j in range(4):
                    for p in range(2):
                        acc += 4 * j + 4
                        end_of[(p, j)] = acc

                filler = []   # list of (gate, key, generator, deadline, weight)

                def add(gate, key, gen, deadline, weight=8):
                    filler.append((gate, key, gen, deadline, weight))

                def block_start(p, j):
                    return end_of[(p, j)] - (4 * j + 4)

                def v_deadline(st):
                    # AV reading VA[st] is first emitted one step after the
                    # step (0, st//4, m=st)
                    return block_start(0, st // 4) + (st % 4) + 1

                # outproj of j-block may only be emitted AFTER normalize(1,j)
                # has been emitted, which happens while processing the step at
                # index end_of[(1,j)] — so its gate is end_of+1.
                add(0, ("qk", 0, 0), qk_proj(0, 0), 0)
                add(0, ("qk", 2, 0), qk_proj(2, 0), 0)
                for st in range(0, 4):
                    add(0, ("v", st), v_proj(st), v_deadline(st))
                add(0, ("qk", 1, 0), qk_proj(1, 0), block_start(1, 0))
                add(0, ("qk", 3, 0), qk_proj(3, 0), block_start(1, 0))
                add(0, ("qk", 0, 1), qk_proj(0, 1), block_start(0, 1))
                add(0, ("qk", 2, 1), qk_proj(2, 1), block_start(0, 1))
                for st in range(4, 8):
                    add(0, ("v", st), v_proj(st), v_deadline(st))
                add(0, ("qk", 1, 1), qk_proj(1, 1), block_start(1, 1))
                add(0, ("qk", 3, 1), qk_proj(3, 1), block_start(1, 1))
                for qm in range(0, 4):
                    add(end_of[(1, 0)] + 1, ("op", qm), out_proj(qm),
                        end_of[(1, 0)] + 2 + 2 * (qm % 4), 4)
                add(0, ("qk", 0, 2), qk_proj(0, 2), block_start(0, 2))
                add(0, ("qk", 2, 2), qk_proj(2, 2), block_start(0, 2))
                for st in range(8, 12):
                    add(0, ("v", st), v_proj(st), v_deadline(st))
                add(0, ("qk", 1, 2), qk_proj(1, 2), block_start(1, 2))
                add(0, ("qk", 3, 2), qk_proj(3, 2), block_start(1, 2))
                for qm in range(4, 8):
                    add(end_of[(1, 1)] + 1, ("op", qm), out_proj(qm),
                        end_of[(1, 1)] + 2 + 2 * (qm % 4), 4)
                add(0, ("qk", 0, 3), qk_proj(0, 3), block_start(0, 3))
                add(0, ("qk", 2, 3), qk_proj(2, 3), block_start(0, 3))
                for st in range(12, 16):
                    add(0, ("v", st), v_proj(st), v_deadline(st))
                add(0, ("qk", 1, 3), qk_proj(1, 3), block_start(1, 3))
                add(0, ("qk", 3, 3), qk_proj(3, 3), block_start(1, 3))
                for qm in range(8, 12):
                    add(end_of[(1, 2)] + 1, ("op", qm), out_proj(qm),
                        end_of[(1, 2)] + 2 + 2 * (qm % 4), 4)
                for qm in range(12, 16):
                    add(end_of[(1, 3)] + 1, ("op", qm), out_proj(qm),
                        n_steps + 1, 4)

                # piecewise-linear emission target: by the start of step
                # `deadline` the unit must be fully emitted; spread the work
                # evenly over the steps before it
                pts = []
                cum = 0
                dmax = 0
                for gate, key, gen, dl, w in filler:
                    dmax = max(dmax, dl)
                    cum += w
                    pts.append((dmax, cum))
                target_at = [0.0] * (n_steps + 2)
                prev_d, prev_c = 0, 0.0
                for dl, c in pts:
                    if dl > prev_d:
                        for s in range(prev_d, min(dl, n_steps + 2)):
                            target_at[s] = prev_c + (c - prev_c) * (s - prev_d) / (dl - prev_d)
                        prev_d, prev_c = dl, float(c)
                    else:
                        prev_c = float(c)
                for s in range(prev_d, n_steps + 2):
                    target_at[s] = prev_c

                fill_state = {"emitted": 0, "idx": 0}
                produced = set()

                def drain_filler(step_idx, budget):
                    done = 0
                    while done < budget and fill_state["idx"] < len(filler):
                        gate, key, gen, _dl, _w = filler[fill_state["idx"]]
                        if gate > step_idx:
                            break
                        try:
                            next(gen)
                            done += 1
                            fill_state["emitted"] += 1
                        except StopIteration:
                            produced.add(key)
                            fill_state["idx"] += 1
                    return done

                def require(step_idx, *keys):
                    # force-drain filler (in order, respecting gates) until
                    # the named units have fully emitted
                    while any(k not in produced for k in keys):
                        if drain_filler(step_idx, 1) == 0:
                            raise RuntimeError(f"unsatisfiable requires {keys}")

                def scores_exp(p, j, m):
                    t = m - 4 * j
                    w0 = 128 * t if t > 0 else 0
                    psS = psum.tile([128, 2, 512], F32, tag="s", bufs=2, name="psS")
                    nc.tensor.matmul(
                        out=psS[:, 0, w0:512],
                        lhsT=KT[p][0:64, 128 * m:128 * (m + 1)],
                        rhs=QT[p][0:64, 512 * j + w0:512 * (j + 1)],
                        start=True, stop=True,
                        tile_position=(0, 0),
                    )
                    nc.tensor.matmul(
                        out=psS[:, 1, w0:512],
                        lhsT=KT[p][64:128, 128 * m:128 * (m + 1)],
                        rhs=QT[p][64:128, 512 * j + w0:512 * (j + 1)],
                        start=True, stop=True,
                        tile_position=(64, 0),
                    )
                    e = work.tile([128, 2, 512], BF16, tag="e", bufs=3, name="e")
                    nc.scalar.activation(
                        e[:, :, w0:512], psS[:, :, w0:512], Exp, scale=0.125)
                    if t >= 0:
                        # causal band: keep where col - key >= 0 (both heads)
                        nc.gpsimd.affine_select(
                            out=e[:, :, w0:w0 + 128],
                            in_=e[:, :, w0:w0 + 128],
                            pattern=[[0, 2], [1, 128]],
                            channel_multiplier=-1,
                            base=0,
                            compare_op=is_ge,
                            fill=0.0,
                        )
                    return e, w0

                def av(acc, p, j, m, e, lo):
                    mlast = 4 * j + 3
                    nc.tensor.matmul(
                        out=acc[:, 0, lo:512],
                        lhsT=VA[:, p, m, 0, :, :],
                        rhs=e[:, 0, lo:512],
                        start=(m == 0), stop=(m == mlast),
                    )
                    nc.tensor.matmul(
                        out=acc[:, 1, lo:512],
                        lhsT=VA[:, p, m, 1, :, :],
                        rhs=e[:, 1, lo:512],
                        start=(m == 0), stop=(m == mlast),
                    )

                def normalize(acc, p, j, nsplit=1):
                    # ctx[v, q] = acc[v, q] / acc[64+v, q] for both heads
                    w = 512 // nsplit
                    for h in range(nsplit):
                        lo, hi = w * h, w * (h + 1)
                        sums = work.tile([64, 2, w], F32, tag="sums", bufs=2, name="sums")
                        nc.vector.tensor_copy(out=sums[:], in_=acc[64:128, :, lo:hi])
                        rec = work.tile([64, 2, w], F32, tag="rec", bufs=2, name="rec")
                        nc.vector.reciprocal_approx_fast(rec[:], sums[:])
                        nc.vector.tensor_tensor(
                            out=ctxp[p][0:64, 512 * j + lo:512 * j + hi],
                            in0=acc[0:64, 0, lo:hi],
                            in1=rec[:, 0, :],
                            op=mult,
                        )
                        codd = work.tile([64, w], BF16, tag="codd", bufs=2, name="codd")
                        nc.vector.tensor_tensor(
                            out=codd[:], in0=acc[0:64, 1, lo:hi], in1=rec[:, 1, :],
                            op=mult)
                        nc.vector.tensor_copy(
                            out=ctxp[p][64:128, 512 * j + lo:512 * j + hi], in_=codd[:])

                # software-pipelined main loop: AV(k-1) is emitted after
                # scores(k) so the PE never head-blocks on exp(k-1); filler
                # (proj / outproj) matmuls pace in to keep the PE dense.
                cur_acc = None
                pend = None   # (acc, p, j, m, e, lo)
                for idx, (p, j, m) in enumerate(steps):
                    if m == 0:
                        # new block: fresh accumulator (WAR on previous
                        # block's normalize is absorbed by boundary filler)
                        cur_acc = psum.tile(
                            [128, 2, 512], F32, tag="acc", bufs=1, name="acc")
                        drain_filler(idx, 4)
                        # Q pair of this block and K pair cols up to 512(j+1)
                        # must be fully emitted before its scores
                        require(idx, ("qk", p, j), ("qk", 2 + p, j))
                    e, w0 = scores_exp(p, j, m)
                    import math
                    need = max(0, math.ceil(target_at[idx + 1]) - fill_state["emitted"])
                    budget = max(2, need)
                    drain_filler(idx, max(1, budget // 2))
                    if pend is not None:
                        pacc, pp, pj, pm, pe, plo = pend
                        require(idx, ("v", pm))
                        av(pacc, pp, pj, pm, pe, plo)
                        if pm == 4 * pj + 3:
                            normalize(pacc, pp, pj)
                    pend = (cur_acc, p, j, m, e, w0)
                    drain_filler(idx, budget - budget // 2)
                pacc, pp, pj, pm, pe, plo = pend
                require(n_steps, ("v", pm))
                av(pacc, pp, pj, pm, pe, plo)
                # split so the tail outproj can start on the first half early
                normalize(pacc, pp, pj, nsplit=2)
                # drain any remaining filler (final outproj blocks)
                while fill_state["idx"] < len(filler):
                    if drain_filler(n_steps + 1, 1 << 30) == 0:
                        break

    nc.finalize()
    return nc


def kernel(x, qkv_w, qkv_b, out_w, out_b):
    from concourse import bass_utils
    import ml_dtypes
    global last_exec_time_ns

    BF = ml_dtypes.bfloat16

    x = np.asarray(x, dtype=np.float32)
    qkv_w = np.asarray(qkv_w, dtype=np.float32)
    qkv_b = np.asarray(qkv_b, dtype=np.float32)
    out_w = np.asarray(out_w, dtype=np.float32)
    out_b = np.asarray(out_b, dtype=np.float32)

    has_qkvb = bool(np.any(qkv_b))
    if has_qkvb not in _cache:
        _cache[has_qkvb] = _build(has_qkvb)
    nc = _cache[has_qkvb]

    in_maps = []
    for c in range(N_CORES):
        b, hg = divmod(c, HG)
        xT = np.ascontiguousarray(x[b].T.astype(BF))
        rows = np.concatenate([
            qkv_w[DL * hg:DL * (hg + 1)],
            qkv_w[D_MODEL + DL * hg:D_MODEL + DL * (hg + 1)],
            qkv_w[2 * D_MODEL + DL * hg:2 * D_MODEL + DL * (hg + 1)],
        ], axis=0)
        wqkvT = np.ascontiguousarray(rows.T.astype(BF))
        woT = np.ascontiguousarray(out_w[:, DL * hg:DL * (hg + 1)].T.astype(BF))
        m = {"xT": xT, "wqkvT": wqkvT, "woT": woT}
        if has_qkvb:
            m["qb"] = np.concatenate([
                qkv_b[DL * hg:DL * (hg + 1)],
                qkv_b[D_MODEL + DL * hg:D_MODEL + DL * (hg + 1)],
                qkv_b[2 * D_MODEL + DL * hg:2 * D_MODEL + DL * (hg + 1)],
            ]).reshape(1, 3 * DL).astype(BF)
        in_maps.append(m)

    res = bass_utils.run_bass_kernel_spmd(nc, in_maps, core_ids=list(range(N_CORES)))
    last_exec_time_ns = res.exec_time_ns

    out = np.zeros((B, S, D_MODEL), dtype=np.float32)
    for c in range(N_CORES):
        b, hg = divmod(c, HG)
        out[b] += np.asarray(res.results[c]["out"], dtype=np.float32)
    out += out_b[None, None, :]
    return out


# revision 21
# speedup vs baseline: 1.0294x; 1.0294x over previous
import sys

sys.path.insert(0, "/opt/trn_rl_repo")

import numpy as np

D_MODEL = 1024
NUM_HEADS = 16
HEAD_DIM = 64
B = 2
S = 2048
N_CORES = 8
HG = 4          # head-groups (cores per batch)
HPC = 4         # heads per core
DL = 256        # local feature width per core (HPC * HEAD_DIM)

_cache = {}
last_exec_time_ns = None


def _build(has_qkvb):
    import concourse.bacc as bacc
    import concourse.mybir as mybir
    import concourse.tile as tile

    F32 = mybir.dt.float32
    BF16 = mybir.dt.bfloat16
    Exp = mybir.ActivationFunctionType.Exp
    mult = mybir.AluOpType.mult
    is_ge = mybir.AluOpType.is_ge

    nc = bacc.Bacc("TRN2", target_bir_lowering=False, debug=False)
    xT_d = nc.dram_tensor("xT", (D_MODEL, S), BF16, kind="ExternalInput")
    wq_d = nc.dram_tensor("wqkvT", (D_MODEL, 3 * DL), BF16, kind="ExternalInput")
    wo_d = nc.dram_tensor("woT", (DL, D_MODEL), BF16, kind="ExternalInput")
    if has_qkvb:
        qb_d = nc.dram_tensor("qb", (1, 3 * DL), BF16, kind="ExternalInput")
    out_d = nc.dram_tensor("out", (S, D_MODEL), F32, kind="ExternalOutput")

    with tile.TileContext(nc) as tc:
        with tc.tile_pool(name="persist", bufs=1) as persist:
            xt = [persist.tile([128, S], BF16, name=f"xt{i}") for i in range(8)]
            wq = [persist.tile([128, 3 * DL], BF16, name=f"wq{i}") for i in range(8)]
            # Q/K packed per head-pair p: partitions 0:64 head 2p, 64:128 head 2p+1
            QT = [persist.tile([128, S], BF16, name=f"QT{p}") for p in range(2)]
            KT = [persist.tile([128, S], BF16, name=f"KT{p}") for p in range(2)]
            # V augmented, single tile: [pair, st, head-parity, (v|ones), 64]
            VA = persist.tile([128, 2, 16, 2, 2, 64], BF16, name="VA")
            # ctx pair-packed: head 2p at partitions 0:64, head 2p+1 at 64:128
            ctxp = [persist.tile([128, S], BF16, name=f"ctxp{p}") for p in range(2)]
            wop = [persist.tile([128, D_MODEL], BF16, name=f"wop{p}") for p in range(2)]

            # input DMAs spread across engine queues for issue parallelism
            # first 512 cols of x land first so the n=0 projection group is
            # not serialized behind the full x transfer; spread across the
            # three DMA-capable queues (sync, gpsimd, scalar)
            qeng = [nc.sync, nc.sync, nc.sync, nc.gpsimd,
                    nc.gpsimd, nc.gpsimd, nc.scalar, nc.scalar]
            for i in range(8):
                qeng[i].dma_start(out=wq[i][:], in_=wq_d[128 * i:128 * (i + 1), :])
                qeng[i].dma_start(
                    out=xt[i][:, 0:512], in_=xT_d[128 * i:128 * (i + 1), 0:512])
            for p in range(2):
                nc.scalar.dma_start(out=wop[p][:], in_=wo_d[128 * p:128 * (p + 1), :])
            for i in range(8):
                qeng[i].dma_start(
                    out=xt[i][:, 512:1024], in_=xT_d[128 * i:128 * (i + 1), 512:1024])
            for i in range(4):
                nc.sync.dma_start(
                    out=xt[i][:, 1024:2048],
                    in_=xT_d[128 * i:128 * (i + 1), 1024:2048])
            for i in range(4, 8):
                nc.gpsimd.dma_start(
                    out=xt[i][:, 1024:2048],
                    in_=xT_d[128 * i:128 * (i + 1), 1024:2048])

            # ones columns of VA (v columns are overwritten by v_proj copies)
            nc.vector.memset(VA[:], 1.0)

            with tc.tile_pool(name="work", bufs=1) as work, \
                 tc.tile_pool(name="psum", bufs=1, space="PSUM") as psum:

                if has_qkvb:
                    qb_t = persist.tile([1, 3 * DL], BF16, name="qb_t")
                    nc.sync.dma_start(out=qb_t[:], in_=qb_d[:])
                    ones_t = persist.tile([1, 512], BF16, name="ones_t")
                    nc.vector.memset(ones_t[:], 1.0)

                # ACT exp-table preload during DMA wait
                warm = work.tile([1, 16], F32, name="warm")
                nc.vector.memset(warm[:], 0.0)
                nc.scalar.activation(warm[:], warm[:], Exp, scale=1.0)

                # ---- filler emitters (each yields per-matmul granularity) ----

                def qk_proj(mi, n):
                    # psq [128 qk-dims, 512 keys]; mi 0,1 = Q pairs; 2,3 = K pairs
                    dst = QT[mi] if mi < 2 else KT[mi - 2]
                    psq = psum.tile([128, 512], F32, tag="p", bufs=2, name="psq")
                    for i in range(8):
                        yield
                        nc.tensor.matmul(
                            out=psq[:],
                            lhsT=wq[i][:, 128 * mi:128 * (mi + 1)],
                            rhs=xt[i][:, 512 * n:512 * (n + 1)],
                            start=(i == 0),
                            stop=(i == 7 and not has_qkvb),
                        )
                    if has_qkvb:
                        nc.tensor.matmul(
                            out=psq[:],
                            lhsT=qb_t[0:1, 128 * mi:128 * (mi + 1)],
                            rhs=ones_t[0:1, :],
                            start=False, stop=True,
                        )
                    nc.vector.tensor_copy(out=dst[:, 512 * n:512 * (n + 1)], in_=psq[:])

                def v_proj(st):
                    # psv [128 keys, (pair, parity, 64)]
                    psv = psum.tile([128, 2, 2, 64], F32, tag="p", bufs=2, name="psv")
                    for i in range(8):
                        yield
                        nc.tensor.matmul(
                            out=psv[:],
                            lhsT=xt[i][:, 128 * st:128 * (st + 1)],
                            rhs=wq[i][:, 512:768],
                            start=(i == 0),
                            stop=(i == 7 and not has_qkvb),
                        )
                    if has_qkvb:
                        nc.tensor.matmul(
                            out=psv[:],
                            lhsT=ones_t[0:1, 0:128],
                            rhs=qb_t[0:1, 512:768],
                            start=False, stop=True,
                        )
                    nc.vector.tensor_copy(out=VA[:, :, st, :, 0, :], in_=psv[:])

                def out_proj(qm):
                    stage = work.tile([128, D_MODEL], F32, tag="st", bufs=2, name="stage")
                    for nh in range(2):
                        pso = psum.tile([128, 512], F32, tag="p", bufs=2, name="pso")
                        for p in range(2):
                            yield
                            nc.tensor.matmul(
                                out=pso[:],
                                lhsT=ctxp[p][:, 128 * qm:128 * (qm + 1)],
                                rhs=wop[p][:, 512 * nh:512 * (nh + 1)],
                                start=(p == 0), stop=(p == 1),
                            )
                        nc.vector.tensor_copy(
                            out=stage[:, 512 * nh:512 * (nh + 1)], in_=pso[:])
                    nc.sync.dma_start(out=out_d[128 * qm:128 * (qm + 1), :], in_=stage[:])

                # global attention step sequence and filler queue with gates.
                # gate = number of attention steps that must be EMITTED before
                # the filler unit may start (keeps FIFO order consistent with
                # data deps, e.g. out_proj needs its ctx block normalized).
                steps = [(p, j, m) for j in range(4) for p in range(2)
                         for m in range(4 * j + 4)]
                n_steps = len(steps)   # 80
                # step index right after block (p,j) finishes:
                end_of = {}
                acc = 0
                for j in range(4):
                    for p in range(2):
                        acc += 4 * j + 4
                        end_of[(p, j)] = acc

                filler = []   # list of (gate, key, generator, deadline, weight)

                def add(gate, key, gen, deadline, weight=8):
                    filler.append((gate, key, gen, deadline, weight))

                def block_start(p, j):
                    return end_of[(p, j)] - (4 * j + 4)

                def v_deadline(st):
                    # AV reading VA[st] is first emitted one step after the
                    # step (0, st//4, m=st)
                    return block_start(0, st // 4) + (st % 4) + 1

                # outproj of j-block may only be emitted AFTER normalize(1,j)
                # has been emitted, which happens while processing the step at
                # index end_of[(1,j)] — so its gate is end_of+1.
                add(0, ("qk", 0, 0), qk_proj(0, 0), 0)
                add(0, ("qk", 2, 0), qk_proj(2, 0), 0)
                for st in range(0, 4):
                    add(0, ("v", st), v_proj(st), v_deadline(st))
                add(0, ("qk", 1, 0), qk_proj(1, 0), block_start(1, 0))
                add(0, ("qk", 3, 0), qk_proj(3, 0), block_start(1, 0))
                add(0, ("qk", 0, 1), qk_proj(0, 1), block_start(0, 1))
                add(0, ("qk", 2, 1), qk_proj(2, 1), block_start(0, 1))
                for st in range(4, 8):
                    add(0, ("v", st), v_proj(st), v_deadline(st))
                add(0, ("qk", 1, 1), qk_proj(1, 1), block_start(1, 1))
                add(0, ("qk", 3, 1), qk_proj(3, 1), block_start(1, 1))
                for qm in range(0, 4):
                    add(end_of[(1, 0)] + 2, ("op", qm), out_proj(qm),
                        end_of[(1, 0)] + 3 + 2 * (qm % 4), 4)
                add(0, ("qk", 0, 2), qk_proj(0, 2), block_start(0, 2))
                add(0, ("qk", 2, 2), qk_proj(2, 2), block_start(0, 2))
                for st in range(8, 12):
                    add(0, ("v", st), v_proj(st), v_deadline(st))
                add(0, ("qk", 1, 2), qk_proj(1, 2), block_start(1, 2))
                add(0, ("qk", 3, 2), qk_proj(3, 2), block_start(1, 2))
                for qm in range(4, 8):
                    add(end_of[(1, 1)] + 2, ("op", qm), out_proj(qm),
                        end_of[(1, 1)] + 3 + 2 * (qm % 4), 4)
                add(0, ("qk", 0, 3), qk_proj(0, 3), block_start(0, 3))
                add(0, ("qk", 2, 3), qk_proj(2, 3), block_start(0, 3))
                for st in range(12, 16):
                    add(0, ("v", st), v_proj(st), v_deadline(st))
                add(0, ("qk", 1, 3), qk_proj(1, 3), block_start(1, 3))
                add(0, ("qk", 3, 3), qk_proj(3, 3), block_start(1, 3))
                for qm in range(8, 12):
                    add(end_of[(1, 2)] + 2, ("op", qm), out_proj(qm),
                        end_of[(1, 2)] + 3 + 2 * (qm % 4), 4)
                for qm in range(12, 16):
                    add(end_of[(1, 3)] + 1, ("op", qm), out_proj(qm),
                        n_steps + 1, 4)

                # piecewise-linear emission target: by the start of step
                # `deadline` the unit must be fully emitted; spread the work
                # evenly over the steps before it
                pts = []
                cum = 0
                dmax = 0
                for gate, key, gen, dl, w in filler:
                    dmax = max(dmax, dl)
                    cum += w
                    pts.append((dmax, cum))
                target_at = [0.0] * (n_steps + 2)
                prev_d, prev_c = 0, 0.0
                for dl, c in pts:
                    if dl > prev_d:
                        for s in range(prev_d, min(dl, n_steps + 2)):
                            target_at[s] = prev_c + (c - prev_c) * (s - prev_d) / (dl - prev_d)
                        prev_d, prev_c = dl, float(c)
                    else:
                        prev_c = float(c)
                for s in range(prev_d, n_steps + 2):
                    target_at[s] = prev_c

                fill_state = {"emitted": 0, "idx": 0}
                produced = set()

                def drain_filler(step_idx, budget, allow_op=True):
                    done = 0
                    while done < budget and fill_state["idx"] < len(filler):
                        gate, key, gen, _dl, _w = filler[fill_state["idx"]]
                        if gate > step_idx:
                            break
                        if not allow_op and key[0] == "op":
                            # outproj blocks on the previous normalize; it may
                            # only enter the PE queue after the pending AV
                            break
                        try:
                            next(gen)
                            done += 1
                            fill_state["emitted"] += 1
                        except StopIteration:
                            produced.add(key)
                            fill_state["idx"] += 1
                    return done

                def require(step_idx, *keys):
                    # force-drain filler (in order, respecting gates) until
                    # the named units have fully emitted
                    while any(k not in produced for k in keys):
                        if drain_filler(step_idx, 1) == 0:
                            raise RuntimeError(f"unsatisfiable requires {keys}")

                def scores_exp(p, j, m):
                    t = m - 4 * j
                    w0 = 128 * t if t > 0 else 0
                    psS = psum.tile([128, 2, 512], F32, tag="s", bufs=2, name="psS")
                    nc.tensor.matmul(
                        out=psS[:, 0, w0:512],
                        lhsT=KT[p][0:64, 128 * m:128 * (m + 1)],
                        rhs=QT[p][0:64, 512 * j + w0:512 * (j + 1)],
                        start=True, stop=True,
                        tile_position=(0, 0),
                    )
                    nc.tensor.matmul(
                        out=psS[:, 1, w0:512],
                        lhsT=KT[p][64:128, 128 * m:128 * (m + 1)],
                        rhs=QT[p][64:128, 512 * j + w0:512 * (j + 1)],
                        start=True, stop=True,
                        tile_position=(64, 0),
                    )
                    e = work.tile([128, 2, 512], BF16, tag="e", bufs=3, name="e")
                    nc.scalar.activation(
                        e[:, :, w0:512], psS[:, :, w0:512], Exp, scale=0.125)
                    if t >= 0:
                        # causal band: keep where col - key >= 0 (both heads)
                        nc.gpsimd.affine_select(
                            out=e[:, :, w0:w0 + 128],
                            in_=e[:, :, w0:w0 + 128],
                            pattern=[[0, 2], [1, 128]],
                            channel_multiplier=-1,
                            base=0,
                            compare_op=is_ge,
                            fill=0.0,
                        )
                    return e, w0

                def av(acc, p, j, m, e, lo):
                    mlast = 4 * j + 3
                    nc.tensor.matmul(
                        out=acc[:, 0, lo:512],
                        lhsT=VA[:, p, m, 0, :, :],
                        rhs=e[:, 0, lo:512],
                        start=(m == 0), stop=(m == mlast),
                    )
                    nc.tensor.matmul(
                        out=acc[:, 1, lo:512],
                        lhsT=VA[:, p, m, 1, :, :],
                        rhs=e[:, 1, lo:512],
                        start=(m == 0), stop=(m == mlast),
                    )

                def normalize(acc, p, j, nsplit=1):
                    # ctx[v, q] = acc[v, q] / acc[64+v, q] for both heads
                    w = 512 // nsplit
                    for h in range(nsplit):
                        lo, hi = w * h, w * (h + 1)
                        sums = work.tile([64, 2, w], F32, tag="sums", bufs=2, name="sums")
                        nc.vector.tensor_copy(out=sums[:], in_=acc[64:128, :, lo:hi])
                        rec = work.tile([64, 2, w], F32, tag="rec", bufs=2, name="rec")
                        nc.vector.reciprocal_approx_fast(rec[:], sums[:])
                        nc.vector.tensor_tensor(
                            out=ctxp[p][0:64, 512 * j + lo:512 * j + hi],
                            in0=acc[0:64, 0, lo:hi],
                            in1=rec[:, 0, :],
                            op=mult,
                        )
                        codd = work.tile([64, w], BF16, tag="codd", bufs=2, name="codd")
                        nc.vector.tensor_tensor(
                            out=codd[:], in0=acc[0:64, 1, lo:hi], in1=rec[:, 1, :],
                            op=mult)
                        nc.vector.tensor_copy(
                            out=ctxp[p][64:128, 512 * j + lo:512 * j + hi], in_=codd[:])

                # software-pipelined main loop: AV(k-1) is emitted after
                # scores(k) so the PE never head-blocks on exp(k-1); filler
                # (proj / outproj) matmuls pace in to keep the PE dense.
                cur_acc = None
                pend = None   # (acc, p, j, m, e, lo)
                for idx, (p, j, m) in enumerate(steps):
                    if m == 0:
                        # new block: fresh accumulator (WAR on previous
                        # block's normalize is absorbed by boundary filler)
                        cur_acc = psum.tile(
                            [128, 2, 512], F32, tag="acc", bufs=1, name="acc")
                        drain_filler(idx, 4)
                        # Q pair of this block and K pair cols up to 512(j+1)
                        # must be fully emitted before its scores
                        require(idx, ("qk", p, j), ("qk", 2 + p, j))
                    e, w0 = scores_exp(p, j, m)
                    import math
                    need = max(0, math.ceil(target_at[idx + 1]) - fill_state["emitted"])
                    budget = max(2, need)
                    drain_filler(idx, max(1, budget // 2), allow_op=False)
                    if pend is not None:
                        pacc, pp, pj, pm, pe, plo = pend
                        require(idx, ("v", pm))
                        av(pacc, pp, pj, pm, pe, plo)
                        if pm == 4 * pj + 3:
                            normalize(pacc, pp, pj)
                    pend = (cur_acc, p, j, m, e, w0)
                    drain_filler(idx, budget - budget // 2)
                pacc, pp, pj, pm, pe, plo = pend
                require(n_steps, ("v", pm))
                av(pacc, pp, pj, pm, pe, plo)
                # split so the tail outproj can start on the first half early
                normalize(pacc, pp, pj, nsplit=2)
                # drain any remaining filler (final outproj blocks)
                while fill_state["idx"] < len(filler):
                    if drain_filler(n_steps + 1, 1 << 30) == 0:
                        break

    nc.finalize()
    return nc


def kernel(x, qkv_w, qkv_b, out_w, out_b):
    from concourse import bass_utils
    import ml_dtypes
    global last_exec_time_ns

    BF = ml_dtypes.bfloat16

    x = np.asarray(x, dtype=np.float32)
    qkv_w = np.asarray(qkv_w, dtype=np.float32)
    qkv_b = np.asarray(qkv_b, dtype=np.float32)
    out_w = np.asarray(out_w, dtype=np.float32)
    out_b = np.asarray(out_b, dtype=np.float32)

    has_qkvb = bool(np.any(qkv_b))
    if has_qkvb not in _cache:
        _cache[has_qkvb] = _build(has_qkvb)
    nc = _cache[has_qkvb]

    in_maps = []
    for c in range(N_CORES):
        b, hg = divmod(c, HG)
        xT = np.ascontiguousarray(x[b].T.astype(BF))
        rows = np.concatenate([
            qkv_w[DL * hg:DL * (hg + 1)],
            qkv_w[D_MODEL + DL * hg:D_MODEL + DL * (hg + 1)],
            qkv_w[2 * D_MODEL + DL * hg:2 * D_MODEL + DL * (hg + 1)],
        ], axis=0)
        wqkvT = np.ascontiguousarray(rows.T.astype(BF))
        woT = np.ascontiguousarray(out_w[:, DL * hg:DL * (hg + 1)].T.astype(BF))
        m = {"xT": xT, "wqkvT": wqkvT, "woT": woT}
        if has_qkvb:
            m["qb"] = np.concatenate([
                qkv_b[DL * hg:DL * (hg + 1)],
                qkv_b[D_MODEL + DL * hg:D_MODEL + DL * (hg + 1)],
                qkv_b[2 * D_MODEL + DL * hg:2 * D_MODEL + DL * (hg + 1)],
            ]).reshape(1, 3 * DL).astype(BF)
        in_maps.append(m)

    res = bass_utils.run_bass_kernel_spmd(nc, in_maps, core_ids=list(range(N_CORES)))
    last_exec_time_ns = res.exec_time_ns

    out = np.zeros((B, S, D_MODEL), dtype=np.float32)
    for c in range(N_CORES):
        b, hg = divmod(c, HG)
        out[b] += np.asarray(res.results[c]["out"], dtype=np.float32)
    out += out_b[None, None, :]
    return out


# revision 24
# speedup vs baseline: 1.0389x; 1.0093x over previous
import sys

sys.path.insert(0, "/opt/trn_rl_repo")

import numpy as np

D_MODEL = 1024
NUM_HEADS = 16
HEAD_DIM = 64
B = 2
S = 2048
N_CORES = 8
HG = 4          # head-groups (cores per batch)
HPC = 4         # heads per core
DL = 256        # local feature width per core (HPC * HEAD_DIM)

_cache = {}
last_exec_time_ns = None


def _build(has_qkvb):
    import concourse.bacc as bacc
    import concourse.mybir as mybir
    import concourse.tile as tile

    F32 = mybir.dt.float32
    BF16 = mybir.dt.bfloat16
    Exp = mybir.ActivationFunctionType.Exp
    mult = mybir.AluOpType.mult
    is_ge = mybir.AluOpType.is_ge

    nc = bacc.Bacc("TRN2", target_bir_lowering=False, debug=False)
    xT_d = nc.dram_tensor("xT", (D_MODEL, S), BF16, kind="ExternalInput")
    wq_d = nc.dram_tensor("wqkvT", (D_MODEL, 3 * DL), BF16, kind="ExternalInput")
    wo_d = nc.dram_tensor("woT", (DL, D_MODEL), BF16, kind="ExternalInput")
    if has_qkvb:
        qb_d = nc.dram_tensor("qb", (1, 3 * DL), BF16, kind="ExternalInput")
    out_d = nc.dram_tensor("out", (S, D_MODEL), F32, kind="ExternalOutput")
    # tiny scratch sink keeping the tail HAM-warming chain live
    scr_d = nc.dram_tensor("scr", (1, 16), F32, kind="ExternalOutput")

    with tile.TileContext(nc) as tc:
        with tc.tile_pool(name="persist", bufs=1) as persist:
            xt = [persist.tile([128, S], BF16, name=f"xt{i}") for i in range(8)]
            wq = [persist.tile([128, 3 * DL], BF16, name=f"wq{i}") for i in range(8)]
            # Q/K packed per head-pair p: partitions 0:64 head 2p, 64:128 head 2p+1
            QT = [persist.tile([128, S], BF16, name=f"QT{p}") for p in range(2)]
            KT = [persist.tile([128, S], BF16, name=f"KT{p}") for p in range(2)]
            # V augmented, single tile: [pair, st, head-parity, (v|ones), 64]
            VA = persist.tile([128, 2, 16, 2, 2, 64], BF16, name="VA")
            # ctx pair-packed: head 2p at partitions 0:64, head 2p+1 at 64:128
            ctxp = [persist.tile([128, S], BF16, name=f"ctxp{p}") for p in range(2)]
            wop = [persist.tile([128, D_MODEL], BF16, name=f"wop{p}") for p in range(2)]

            # input DMAs spread across engine queues for issue parallelism
            # first 512 cols of x land first so the n=0 projection group is
            # not serialized behind the full x transfer; spread across the
            # three DMA-capable queues (sync, gpsimd, scalar)
            qeng = [nc.sync, nc.sync, nc.sync, nc.gpsimd,
                    nc.gpsimd, nc.gpsimd, nc.scalar, nc.scalar]
            for i in range(8):
                qeng[i].dma_start(out=wq[i][:], in_=wq_d[128 * i:128 * (i + 1), :])
                qeng[i].dma_start(
                    out=xt[i][:, 0:512], in_=xT_d[128 * i:128 * (i + 1), 0:512])
            for p in range(2):
                nc.scalar.dma_start(out=wop[p][:], in_=wo_d[128 * p:128 * (p + 1), :])
            for i in range(8):
                qeng[i].dma_start(
                    out=xt[i][:, 512:1024], in_=xT_d[128 * i:128 * (i + 1), 512:1024])
            for i in range(4):
                nc.sync.dma_start(
                    out=xt[i][:, 1024:2048],
                    in_=xT_d[128 * i:128 * (i + 1), 1024:2048])
            for i in range(4, 8):
                nc.gpsimd.dma_start(
                    out=xt[i][:, 1024:2048],
                    in_=xT_d[128 * i:128 * (i + 1), 1024:2048])

            # ones columns of VA (v columns are overwritten by v_proj copies)
            nc.vector.memset(VA[:], 1.0)

            with tc.tile_pool(name="work", bufs=1) as work, \
                 tc.tile_pool(name="psum", bufs=1, space="PSUM") as psum:

                if has_qkvb:
                    qb_t = persist.tile([1, 3 * DL], BF16, name="qb_t")
                    nc.sync.dma_start(out=qb_t[:], in_=qb_d[:])
                    ones_t = persist.tile([1, 512], BF16, name="ones_t")
                    nc.vector.memset(ones_t[:], 1.0)

                # HAM pre-warm: dummy matmul chain fills the input-DMA wait
                # so the PE clock is at 2.4GHz when real work starts; its
                # result feeds the exp-table preload (keeps the chain live)
                dummy = work.tile([128, 512], BF16, tag="dummy", bufs=1, name="dummy")
                nc.vector.memset(dummy[:], 0.0)
                psd = psum.tile([128, 512], F32, tag="p", bufs=2, name="psd")
                for k in range(28):
                    nc.tensor.matmul(
                        out=psd[:], lhsT=dummy[:, 0:128], rhs=dummy[:],
                        start=(k == 0), stop=(k == 27))
                warm = work.tile([1, 16], F32, name="warm")
                nc.vector.tensor_copy(out=warm[:], in_=psd[0:1, 0:16])
                nc.scalar.activation(warm[:], warm[:], Exp, scale=1.0)

                # ---- filler emitters (each yields per-matmul granularity) ----

                def qk_proj(mi, n):
                    # psq [128 qk-dims, 512 keys]; mi 0,1 = Q pairs; 2,3 = K pairs
                    dst = QT[mi] if mi < 2 else KT[mi - 2]
                    psq = psum.tile([128, 512], F32, tag="p", bufs=2, name="psq")
                    for i in range(8):
                        yield
                        nc.tensor.matmul(
                            out=psq[:],
                            lhsT=wq[i][:, 128 * mi:128 * (mi + 1)],
                            rhs=xt[i][:, 512 * n:512 * (n + 1)],
                            start=(i == 0),
                            stop=(i == 7 and not has_qkvb),
                        )
                    if has_qkvb:
                        nc.tensor.matmul(
                            out=psq[:],
                            lhsT=qb_t[0:1, 128 * mi:128 * (mi + 1)],
                            rhs=ones_t[0:1, :],
                            start=False, stop=True,
                        )
                    nc.vector.tensor_copy(out=dst[:, 512 * n:512 * (n + 1)], in_=psq[:])

                def v_proj(st):
                    # psv [128 keys, (pair, parity, 64)]
                    psv = psum.tile([128, 2, 2, 64], F32, tag="p", bufs=2, name="psv")
                    for i in range(8):
                        yield
                        nc.tensor.matmul(
                            out=psv[:],
                            lhsT=xt[i][:, 128 * st:128 * (st + 1)],
                            rhs=wq[i][:, 512:768],
                            start=(i == 0),
                            stop=(i == 7 and not has_qkvb),
                        )
                    if has_qkvb:
                        nc.tensor.matmul(
                            out=psv[:],
                            lhsT=ones_t[0:1, 0:128],
                            rhs=qb_t[0:1, 512:768],
                            start=False, stop=True,
                        )
                    nc.vector.tensor_copy(out=VA[:, :, st, :, 0, :], in_=psv[:])

                def out_proj(qm):
                    stage = work.tile([128, D_MODEL], F32, tag="st", bufs=2, name="stage")
                    for nh in range(2):
                        pso = psum.tile([128, 512], F32, tag="p", bufs=2, name="pso")
                        for p in range(2):
                            yield
                            nc.tensor.matmul(
                                out=pso[:],
                                lhsT=ctxp[p][:, 128 * qm:128 * (qm + 1)],
                                rhs=wop[p][:, 512 * nh:512 * (nh + 1)],
                                start=(p == 0), stop=(p == 1),
                            )
                        nc.vector.tensor_copy(
                            out=stage[:, 512 * nh:512 * (nh + 1)], in_=pso[:])
                    nc.sync.dma_start(out=out_d[128 * qm:128 * (qm + 1), :], in_=stage[:])

                # global attention step sequence and filler queue with gates.
                # gate = number of attention steps that must be EMITTED before
                # the filler unit may start (keeps FIFO order consistent with
                # data deps, e.g. out_proj needs its ctx block normalized).
                steps = [(p, j, m) for j in range(4) for p in range(2)
                         for m in range(4 * j + 4)]
                n_steps = len(steps)   # 80
                # step index right after block (p,j) finishes:
                end_of = {}
                acc = 0
                for j in range(4):
                    for p in range(2):
                        acc += 4 * j + 4
                        end_of[(p, j)] = acc

                filler = []   # list of (gate, key, generator, deadline, weight)

                def add(gate, key, gen, deadline, weight=8):
                    filler.append((gate, key, gen, deadline, weight))

                def block_start(p, j):
                    return end_of[(p, j)] - (4 * j + 4)

                def v_deadline(st):
                    # AV reading VA[st] is first emitted one step after the
                    # step (0, st//4, m=st)
                    return block_start(0, st // 4) + (st % 4) + 1

                # outproj of j-block may only be emitted AFTER normalize(1,j)
                # has been emitted, which happens while processing the step at
                # index end_of[(1,j)] — so its gate is end_of+1.
                add(0, ("qk", 0, 0), qk_proj(0, 0), 0)
                add(0, ("qk", 2, 0), qk_proj(2, 0), 0)
                for st in range(0, 4):
                    add(0, ("v", st), v_proj(st), v_deadline(st))
                add(0, ("qk", 1, 0), qk_proj(1, 0), block_start(1, 0))
                add(0, ("qk", 3, 0), qk_proj(3, 0), block_start(1, 0))
                add(0, ("qk", 0, 1), qk_proj(0, 1), block_start(0, 1))
                add(0, ("qk", 2, 1), qk_proj(2, 1), block_start(0, 1))
                for st in range(4, 8):
                    add(0, ("v", st), v_proj(st), v_deadline(st))
                add(0, ("qk", 1, 1), qk_proj(1, 1), block_start(1, 1))
                add(0, ("qk", 3, 1), qk_proj(3, 1), block_start(1, 1))
                for qm in range(0, 4):
                    add(end_of[(1, 0)] + 2, ("op", qm), out_proj(qm),
                        end_of[(1, 0)] + 3 + 2 * (qm % 4), 4)
                add(0, ("qk", 0, 2), qk_proj(0, 2), block_start(0, 2))
                add(0, ("qk", 2, 2), qk_proj(2, 2), block_start(0, 2))
                for st in range(8, 12):
                    add(0, ("v", st), v_proj(st), v_deadline(st))
                add(0, ("qk", 1, 2), qk_proj(1, 2), block_start(1, 2))
                add(0, ("qk", 3, 2), qk_proj(3, 2), block_start(1, 2))
                for qm in range(4, 8):
                    add(end_of[(1, 1)] + 2, ("op", qm), out_proj(qm),
                        end_of[(1, 1)] + 3 + 2 * (qm % 4), 4)
                add(0, ("qk", 0, 3), qk_proj(0, 3), block_start(0, 3))
                add(0, ("qk", 2, 3), qk_proj(2, 3), block_start(0, 3))
                for st in range(12, 16):
                    add(0, ("v", st), v_proj(st), v_deadline(st))
                add(0, ("qk", 1, 3), qk_proj(1, 3), block_start(1, 3))
                add(0, ("qk", 3, 3), qk_proj(3, 3), block_start(1, 3))
                for qm in range(8, 12):
                    add(end_of[(1, 2)] + 2, ("op", qm), out_proj(qm),
                        end_of[(1, 2)] + 3 + 2 * (qm % 4), 4)
                for qm in range(12, 16):
                    add(end_of[(1, 3)] + 1, ("op", qm), out_proj(qm),
                        n_steps + 1, 4)

                # piecewise-linear emission target: by the start of step
                # `deadline` the unit must be fully emitted; spread the work
                # evenly over the steps before it
                pts = []
                cum = 0
                dmax = 0
                for gate, key, gen, dl, w in filler:
                    dmax = max(dmax, dl)
                    cum += w
                    pts.append((dmax, cum))
                target_at = [0.0] * (n_steps + 2)
                prev_d, prev_c = 0, 0.0
                for dl, c in pts:
                    if dl > prev_d:
                        for s in range(prev_d, min(dl, n_steps + 2)):
                            target_at[s] = prev_c + (c - prev_c) * (s - prev_d) / (dl - prev_d)
                        prev_d, prev_c = dl, float(c)
                    else:
                        prev_c = float(c)
                for s in range(prev_d, n_steps + 2):
                    target_at[s] = prev_c

                fill_state = {"emitted": 0, "idx": 0}
                produced = set()

                def drain_filler(step_idx, budget, allow_op=True):
                    done = 0
                    while done < budget and fill_state["idx"] < len(filler):
                        gate, key, gen, _dl, _w = filler[fill_state["idx"]]
                        if gate > step_idx:
                            break
                        if not allow_op and key[0] == "op":
                            # outproj blocks on the previous normalize; it may
                            # only enter the PE queue after the pending AV
                            break
                        try:
                            next(gen)
                            done += 1
                            fill_state["emitted"] += 1
                        except StopIteration:
                            produced.add(key)
                            fill_state["idx"] += 1
                    return done

                def require(step_idx, *keys):
                    # force-drain filler (in order, respecting gates) until
                    # the named units have fully emitted
                    while any(k not in produced for k in keys):
                        if drain_filler(step_idx, 1) == 0:
                            raise RuntimeError(f"unsatisfiable requires {keys}")

                def scores_exp(p, j, m):
                    t = m - 4 * j
                    w0 = 128 * t if t > 0 else 0
                    psS = psum.tile([128, 2, 512], F32, tag="s", bufs=2, name="psS")
                    nc.tensor.matmul(
                        out=psS[:, 0, w0:512],
                        lhsT=KT[p][0:64, 128 * m:128 * (m + 1)],
                        rhs=QT[p][0:64, 512 * j + w0:512 * (j + 1)],
                        start=True, stop=True,
                        tile_position=(0, 0),
                    )
                    nc.tensor.matmul(
                        out=psS[:, 1, w0:512],
                        lhsT=KT[p][64:128, 128 * m:128 * (m + 1)],
                        rhs=QT[p][64:128, 512 * j + w0:512 * (j + 1)],
                        start=True, stop=True,
                        tile_position=(64, 0),
                    )
                    e = work.tile([128, 2, 512], BF16, tag="e", bufs=3, name="e")
                    nc.scalar.activation(
                        e[:, :, w0:512], psS[:, :, w0:512], Exp, scale=0.125)
                    if t >= 0:
                        # causal band: keep where col - key >= 0 (both heads)
                        nc.gpsimd.affine_select(
                            out=e[:, :, w0:w0 + 128],
                            in_=e[:, :, w0:w0 + 128],
                            pattern=[[0, 2], [1, 128]],
                            channel_multiplier=-1,
                            base=0,
                            compare_op=is_ge,
                            fill=0.0,
                        )
                    return e, w0

                def av(acc, p, j, m, e, lo):
                    mlast = 4 * j + 3
                    nc.tensor.matmul(
                        out=acc[:, 0, lo:512],
                        lhsT=VA[:, p, m, 0, :, :],
                        rhs=e[:, 0, lo:512],
                        start=(m == 0), stop=(m == mlast),
                    )
                    nc.tensor.matmul(
                        out=acc[:, 1, lo:512],
                        lhsT=VA[:, p, m, 1, :, :],
                        rhs=e[:, 1, lo:512],
                        start=(m == 0), stop=(m == mlast),
                    )

                def normalize(acc, p, j, nsplit=1):
                    # ctx[v, q] = acc[v, q] / acc[64+v, q] for both heads
                    w = 512 // nsplit
                    for h in range(nsplit):
                        lo, hi = w * h, w * (h + 1)
                        sums = work.tile([64, 2, w], F32, tag="sums", bufs=2, name="sums")
                        nc.vector.tensor_copy(out=sums[:], in_=acc[64:128, :, lo:hi])
                        rec = work.tile([64, 2, w], F32, tag="rec", bufs=2, name="rec")
                        nc.vector.reciprocal_approx_fast(rec[:], sums[:])
                        nc.vector.tensor_tensor(
                            out=ctxp[p][0:64, 512 * j + lo:512 * j + hi],
                            in0=acc[0:64, 0, lo:hi],
                            in1=rec[:, 0, :],
                            op=mult,
                        )
                        codd = work.tile([64, w], BF16, tag="codd", bufs=2, name="codd")
                        nc.vector.tensor_tensor(
                            out=codd[:], in0=acc[0:64, 1, lo:hi], in1=rec[:, 1, :],
                            op=mult)
                        nc.vector.tensor_copy(
                            out=ctxp[p][64:128, 512 * j + lo:512 * j + hi], in_=codd[:])

                # software-pipelined main loop: AV(k-1) is emitted after
                # scores(k) so the PE never head-blocks on exp(k-1); filler
                # (proj / outproj) matmuls pace in to keep the PE dense.
                cur_acc = None
                pend = None   # (acc, p, j, m, e, lo)
                for idx, (p, j, m) in enumerate(steps):
                    if m == 0:
                        # new block: fresh accumulator (WAR on previous
                        # block's normalize is absorbed by boundary filler)
                        cur_acc = psum.tile(
                            [128, 2, 512], F32, tag="acc", bufs=1, name="acc")
                        drain_filler(idx, 4)
                        # Q pair of this block and K pair cols up to 512(j+1)
                        # must be fully emitted before its scores
                        require(idx, ("qk", p, j), ("qk", 2 + p, j))
                    e, w0 = scores_exp(p, j, m)
                    import math
                    need = max(0, math.ceil(target_at[idx + 1]) - fill_state["emitted"])
                    budget = max(2, need)
                    drain_filler(idx, max(1, budget // 2), allow_op=False)
                    if pend is not None:
                        pacc, pp, pj, pm, pe, plo = pend
                        require(idx, ("v", pm))
                        av(pacc, pp, pj, pm, pe, plo)
                        if pm == 4 * pj + 3:
                            normalize(pacc, pp, pj)
                    pend = (cur_acc, p, j, m, e, w0)
                    drain_filler(idx, budget - budget // 2)
                pacc, pp, pj, pm, pe, plo = pend
                require(n_steps, ("v", pm))
                av(pacc, pp, pj, pm, pe, plo)
                # split so the tail outproj can start on the first half early
                normalize(pacc, pp, pj, nsplit=2)
                # keep the PE clock warm across the final normalize latency
                # so the tail outproj runs at full rate
                psd2 = psum.tile([128, 512], F32, tag="p", bufs=2, name="psd")
                for k in range(12):
                    nc.tensor.matmul(
                        out=psd2[:], lhsT=dummy[:, 0:128], rhs=dummy[:],
                        start=(k == 0), stop=(k == 11))
                scr_t = work.tile([1, 16], F32, name="scr_t")
                nc.vector.tensor_copy(out=scr_t[:], in_=psd2[0:1, 0:16])
                nc.sync.dma_start(out=scr_d[:], in_=scr_t[:])
                # drain any remaining filler (final outproj blocks)
                while fill_state["idx"] < len(filler):
                    if drain_filler(n_steps + 1, 1 << 30) == 0:
                        break

    nc.finalize()
    return nc


def kernel(x, qkv_w, qkv_b, out_w, out_b):
    from concourse import bass_utils
    import ml_dtypes
    global last_exec_time_ns

    BF = ml_dtypes.bfloat16

    x = np.asarray(x, dtype=np.float32)
    qkv_w = np.asarray(qkv_w, dtype=np.float32)
    qkv_b = np.asarray(qkv_b, dtype=np.float32)
    out_w = np.asarray(out_w, dtype=np.float32)
    out_b = np.asarray(out_b, dtype=np.float32)

    has_qkvb = bool(np.any(qkv_b))
    if has_qkvb not in _cache:
        _cache[has_qkvb] = _build(has_qkvb)
    nc = _cache[has_qkvb]

    in_maps = []
    for c in range(N_CORES):
        b, hg = divmod(c, HG)
        xT = np.ascontiguousarray(x[b].T.astype(BF))
        rows = np.concatenate([
            qkv_w[DL * hg:DL * (hg + 1)],
            qkv_w[D_MODEL + DL * hg:D_MODEL + DL * (hg + 1)],
            qkv_w[2 * D_MODEL + DL * hg:2 * D_MODEL + DL * (hg + 1)],
        ], axis=0)
        wqkvT = np.ascontiguousarray(rows.T.astype(BF))
        woT = np.ascontiguousarray(out_w[:, DL * hg:DL * (hg + 1)].T.astype(BF))
        m = {"xT": xT, "wqkvT": wqkvT, "woT": woT}
        if has_qkvb:
            m["qb"] = np.concatenate([
                qkv_b[DL * hg:DL * (hg + 1)],
                qkv_b[D_MODEL + DL * hg:D_MODEL + DL * (hg + 1)],
                qkv_b[2 * D_MODEL + DL * hg:2 * D_MODEL + DL * (hg + 1)],
            ]).reshape(1, 3 * DL).astype(BF)
        in_maps.append(m)

    res = bass_utils.run_bass_kernel_spmd(nc, in_maps, core_ids=list(range(N_CORES)))
    last_exec_time_ns = res.exec_time_ns

    out = np.zeros((B, S, D_MODEL), dtype=np.float32)
    for c in range(N_CORES):
        b, hg = divmod(c, HG)
        out[b] += np.asarray(res.results[c]["out"], dtype=np.float32)
    out += out_b[None, None, :]
    return out


# revision 28
# speedup vs baseline: 1.0545x; 1.0150x over previous
import sys

sys.path.insert(0, "/opt/trn_rl_repo")

import numpy as np

D_MODEL = 1024
NUM_HEADS = 16
HEAD_DIM = 64
B = 2
S = 2048
N_CORES = 8
HG = 4          # head-groups (cores per batch)
HPC = 4         # heads per core
DL = 256        # local feature width per core (HPC * HEAD_DIM)

_cache = {}
last_exec_time_ns = None


def _build(has_qkvb):
    import concourse.bacc as bacc
    import concourse.mybir as mybir
    import concourse.tile as tile

    F32 = mybir.dt.float32
    BF16 = mybir.dt.bfloat16
    Exp = mybir.ActivationFunctionType.Exp
    mult = mybir.AluOpType.mult
    is_ge = mybir.AluOpType.is_ge

    nc = bacc.Bacc("TRN2", target_bir_lowering=False, debug=False)
    xT_d = nc.dram_tensor("xT", (D_MODEL, S), BF16, kind="ExternalInput")
    wq_d = nc.dram_tensor("wqkvT", (D_MODEL, 3 * DL), BF16, kind="ExternalInput")
    wo_d = nc.dram_tensor("woT", (DL, D_MODEL), BF16, kind="ExternalInput")
    if has_qkvb:
        qb_d = nc.dram_tensor("qb", (1, 3 * DL), BF16, kind="ExternalInput")
    out_d = nc.dram_tensor("out", (S, D_MODEL), F32, kind="ExternalOutput")
    # tiny scratch sink keeping the tail HAM-warming chain live
    scr_d = nc.dram_tensor("scr", (1, 16), F32, kind="ExternalOutput")

    with tile.TileContext(nc) as tc:
        with tc.tile_pool(name="persist", bufs=1) as persist:
            xt = [persist.tile([128, S], BF16, name=f"xt{i}") for i in range(8)]
            wq = [persist.tile([128, 3 * DL], BF16, name=f"wq{i}") for i in range(8)]
            # Q/K packed per head-pair p: partitions 0:64 head 2p, 64:128 head 2p+1
            QT = [persist.tile([128, S], BF16, name=f"QT{p}") for p in range(2)]
            KT = [persist.tile([128, S], BF16, name=f"KT{p}") for p in range(2)]
            # V augmented, single tile: [pair, st, head-parity, (v|ones), 64]
            VA = persist.tile([128, 2, 16, 2, 2, 64], BF16, name="VA")
            # ctx pair-packed: head 2p at partitions 0:64, head 2p+1 at 64:128
            ctxp = [persist.tile([128, S], BF16, name=f"ctxp{p}") for p in range(2)]
            wop = [persist.tile([128, D_MODEL], BF16, name=f"wop{p}") for p in range(2)]

            # input DMAs spread across engine queues for issue parallelism
            # first 512 cols of x land first so the n=0 projection group is
            # not serialized behind the full x transfer; spread across the
            # three DMA-capable queues (sync, gpsimd, scalar)
            qeng = [nc.sync, nc.sync, nc.sync, nc.gpsimd,
                    nc.gpsimd, nc.gpsimd, nc.scalar, nc.scalar]
            for i in range(8):
                qeng[i].dma_start(
                    out=wq[i][:, 0:384], in_=wq_d[128 * i:128 * (i + 1), 0:384])
                qeng[i].dma_start(
                    out=xt[i][:, 0:512], in_=xT_d[128 * i:128 * (i + 1), 0:512])
            for i in range(8):
                qeng[i].dma_start(
                    out=wq[i][:, 384:768], in_=wq_d[128 * i:128 * (i + 1), 384:768])
            for p in range(2):
                nc.scalar.dma_start(out=wop[p][:], in_=wo_d[128 * p:128 * (p + 1), :])
            for i in range(8):
                qeng[i].dma_start(
                    out=xt[i][:, 512:1024], in_=xT_d[128 * i:128 * (i + 1), 512:1024])
            for i in range(4):
                nc.sync.dma_start(
                    out=xt[i][:, 1024:2048],
                    in_=xT_d[128 * i:128 * (i + 1), 1024:2048])
            for i in range(4, 8):
                nc.gpsimd.dma_start(
                    out=xt[i][:, 1024:2048],
                    in_=xT_d[128 * i:128 * (i + 1), 1024:2048])

            # (VA ones-memset happens after the HAM-warm dummy memset below)

            with tc.tile_pool(name="work", bufs=1) as work, \
                 tc.tile_pool(name="psum", bufs=1, space="PSUM") as psum:

                if has_qkvb:
                    qb_t = persist.tile([1, 3 * DL], BF16, name="qb_t")
                    nc.sync.dma_start(out=qb_t[:], in_=qb_d[:])
                    ones_t = persist.tile([1, 512], BF16, name="ones_t")
                    nc.vector.memset(ones_t[:], 1.0)

                # HAM pre-warm: dummy matmul chain fills the input-DMA wait
                # so the PE clock is at 2.4GHz when real work starts; its
                # result feeds the exp-table preload (keeps the chain live)
                dummy = work.tile([128, 128], BF16, tag="dummy", bufs=1, name="dummy")
                nc.vector.memset(dummy[:], 0.0)
                psd = psum.tile([128, 128], F32, tag="p", bufs=2, name="psd")
                for k in range(64):
                    nc.tensor.matmul(
                        out=psd[:], lhsT=dummy[:], rhs=dummy[:],
                        start=(k == 0), stop=(k == 63))
                warm = work.tile([1, 16], F32, name="warm")
                nc.vector.tensor_copy(out=warm[:], in_=psd[0:1, 0:16])
                nc.scalar.activation(warm[:], warm[:], Exp, scale=1.0)
                # ones columns of VA (v columns are overwritten by v_proj)
                nc.vector.memset(VA[:], 1.0)

                # ---- filler emitters (each yields per-matmul granularity) ----

                def qk_proj(mi, n):
                    # psq [128 qk-dims, 512 keys]; mi 0,1 = Q pairs; 2,3 = K pairs
                    dst = QT[mi] if mi < 2 else KT[mi - 2]
                    psq = psum.tile([128, 512], F32, tag="p", bufs=2, name="psq")
                    for i in range(8):
                        yield
                        nc.tensor.matmul(
                            out=psq[:],
                            lhsT=wq[i][:, 128 * mi:128 * (mi + 1)],
                            rhs=xt[i][:, 512 * n:512 * (n + 1)],
                            start=(i == 0),
                            stop=(i == 7 and not has_qkvb),
                        )
                    if has_qkvb:
                        nc.tensor.matmul(
                            out=psq[:],
                            lhsT=qb_t[0:1, 128 * mi:128 * (mi + 1)],
                            rhs=ones_t[0:1, :],
                            start=False, stop=True,
                        )
                    nc.vector.tensor_copy(out=dst[:, 512 * n:512 * (n + 1)], in_=psq[:])

                def v_proj(st):
                    # psv [128 keys, (pair, parity, 64)]
                    psv = psum.tile([128, 2, 2, 64], F32, tag="p", bufs=2, name="psv")
                    for i in range(8):
                        yield
                        nc.tensor.matmul(
                            out=psv[:],
                            lhsT=xt[i][:, 128 * st:128 * (st + 1)],
                            rhs=wq[i][:, 512:768],
                            start=(i == 0),
                            stop=(i == 7 and not has_qkvb),
                        )
                    if has_qkvb:
                        nc.tensor.matmul(
                            out=psv[:],
                            lhsT=ones_t[0:1, 0:128],
                            rhs=qb_t[0:1, 512:768],
                            start=False, stop=True,
                        )
                    nc.vector.tensor_copy(out=VA[:, :, st, :, 0, :], in_=psv[:])

                def out_proj(qm):
                    stage = work.tile([128, D_MODEL], F32, tag="st", bufs=2, name="stage")
                    for nh in range(2):
                        pso = psum.tile([128, 512], F32, tag="p", bufs=2, name="pso")
                        for p in range(2):
                            yield
                            nc.tensor.matmul(
                                out=pso[:],
                                lhsT=ctxp[p][:, 128 * qm:128 * (qm + 1)],
                                rhs=wop[p][:, 512 * nh:512 * (nh + 1)],
                                start=(p == 0), stop=(p == 1),
                            )
                        nc.vector.tensor_copy(
                            out=stage[:, 512 * nh:512 * (nh + 1)], in_=pso[:])
                    nc.sync.dma_start(out=out_d[128 * qm:128 * (qm + 1), :], in_=stage[:])

                # global attention step sequence and filler queue with gates.
                # gate = number of attention steps that must be EMITTED before
                # the filler unit may start (keeps FIFO order consistent with
                # data deps, e.g. out_proj needs its ctx block normalized).
                steps = [(p, j, m) for j in range(4) for p in range(2)
                         for m in range(4 * j + 4)]
                n_steps = len(steps)   # 80
                # step index right after block (p,j) finishes:
                end_of = {}
                acc = 0
                for j in range(4):
                    for p in range(2):
                        acc += 4 * j + 4
                        end_of[(p, j)] = acc

                filler = []   # list of (gate, key, generator, deadline, weight)

                def add(gate, key, gen, deadline, weight=8):
                    filler.append((gate, key, gen, deadline, weight))

                def block_start(p, j):
                    return end_of[(p, j)] - (4 * j + 4)

                def v_deadline(st):
                    # AV reading VA[st] is first emitted one step after the
                    # step (0, st//4, m=st)
                    return block_start(0, st // 4) + (st % 4) + 1

                # outproj of j-block may only be emitted AFTER normalize(1,j)
                # has been emitted, which happens while processing the step at
                # index end_of[(1,j)] — so its gate is end_of+1.
                add(0, ("qk", 0, 0), qk_proj(0, 0), 0)
                add(0, ("qk", 2, 0), qk_proj(2, 0), 0)
                for st in range(0, 4):
                    add(0, ("v", st), v_proj(st), v_deadline(st))
                add(0, ("qk", 1, 0), qk_proj(1, 0), block_start(1, 0))
                add(0, ("qk", 3, 0), qk_proj(3, 0), block_start(1, 0))
                add(0, ("qk", 0, 1), qk_proj(0, 1), block_start(0, 1))
                add(0, ("qk", 2, 1), qk_proj(2, 1), block_start(0, 1))
                for st in range(4, 8):
                    add(0, ("v", st), v_proj(st), v_deadline(st))
                add(0, ("qk", 1, 1), qk_proj(1, 1), block_start(1, 1))
                add(0, ("qk", 3, 1), qk_proj(3, 1), block_start(1, 1))
                for qm in range(0, 4):
                    add(end_of[(1, 0)] + 2, ("op", qm), out_proj(qm),
                        end_of[(1, 0)] + 3 + 2 * (qm % 4), 4)
                add(0, ("qk", 0, 2), qk_proj(0, 2), block_start(0, 2))
                add(0, ("qk", 2, 2), qk_proj(2, 2), block_start(0, 2))
                for st in range(8, 12):
                    add(0, ("v", st), v_proj(st), v_deadline(st))
                add(0, ("qk", 1, 2), qk_proj(1, 2), block_start(1, 2))
                add(0, ("qk", 3, 2), qk_proj(3, 2), block_start(1, 2))
                for qm in range(4, 8):
                    add(end_of[(1, 1)] + 2, ("op", qm), out_proj(qm),
                        end_of[(1, 1)] + 3 + 2 * (qm % 4), 4)
                add(0, ("qk", 0, 3), qk_proj(0, 3), block_start(0, 3))
                add(0, ("qk", 2, 3), qk_proj(2, 3), block_start(0, 3))
                for st in range(12, 16):
                    add(0, ("v", st), v_proj(st), v_deadline(st))
                add(0, ("qk", 1, 3), qk_proj(1, 3), block_start(1, 3))
                add(0, ("qk", 3, 3), qk_proj(3, 3), block_start(1, 3))
                for qm in range(8, 12):
                    add(end_of[(1, 2)] + 2, ("op", qm), out_proj(qm),
                        end_of[(1, 2)] + 3 + 2 * (qm % 4), 4)
                for qm in range(12, 16):
                    add(end_of[(1, 3)] + 1, ("op", qm), out_proj(qm),
                        n_steps + 1, 4)

                # piecewise-linear emission target: by the start of step
                # `deadline` the unit must be fully emitted; spread the work
                # evenly over the steps before it
                pts = []
                cum = 0
                dmax = 0
                for gate, key, gen, dl, w in filler:
                    dmax = max(dmax, dl)
                    cum += w
                    pts.append((dmax, cum))
                target_at = [0.0] * (n_steps + 2)
                prev_d, prev_c = 0, 0.0
                for dl, c in pts:
                    if dl > prev_d:
                        for s in range(prev_d, min(dl, n_steps + 2)):
                            target_at[s] = prev_c + (c - prev_c) * (s - prev_d) / (dl - prev_d)
                        prev_d, prev_c = dl, float(c)
                    else:
                        prev_c = float(c)
                for s in range(prev_d, n_steps + 2):
                    target_at[s] = prev_c

                fill_state = {"emitted": 0, "idx": 0}
                produced = set()

                def drain_filler(step_idx, budget, allow_op=True):
                    done = 0
                    while done < budget and fill_state["idx"] < len(filler):
                        gate, key, gen, _dl, _w = filler[fill_state["idx"]]
                        if gate > step_idx:
                            break
                        if not allow_op and key[0] == "op":
                            # outproj blocks on the previous normalize; it may
                            # only enter the PE queue after the pending AV
                            break
                        try:
                            next(gen)
                            done += 1
                            fill_state["emitted"] += 1
                        except StopIteration:
                            produced.add(key)
                            fill_state["idx"] += 1
                    return done

                def require(step_idx, *keys):
                    # force-drain filler (in order, respecting gates) until
                    # the named units have fully emitted
                    while any(k not in produced for k in keys):
                        if drain_filler(step_idx, 1) == 0:
                            raise RuntimeError(f"unsatisfiable requires {keys}")

                def scores_exp(p, j, m):
                    t = m - 4 * j
                    w0 = 128 * t if t > 0 else 0
                    psS = psum.tile([128, 2, 512], F32, tag="s", bufs=2, name="psS")
                    nc.tensor.matmul(
                        out=psS[:, 0, w0:512],
                        lhsT=KT[p][0:64, 128 * m:128 * (m + 1)],
                        rhs=QT[p][0:64, 512 * j + w0:512 * (j + 1)],
                        start=True, stop=True,
                        tile_position=(0, 0),
                    )
                    nc.tensor.matmul(
                        out=psS[:, 1, w0:512],
                        lhsT=KT[p][64:128, 128 * m:128 * (m + 1)],
                        rhs=QT[p][64:128, 512 * j + w0:512 * (j + 1)],
                        start=True, stop=True,
                        tile_position=(64, 0),
                    )
                    e = work.tile([128, 2, 512], BF16, tag="e", bufs=3, name="e")
                    nc.scalar.activation(
                        e[:, :, w0:512], psS[:, :, w0:512], Exp, scale=0.125)
                    if t >= 0:
                        # causal band: keep where col - key >= 0 (both heads)
                        nc.gpsimd.affine_select(
                            out=e[:, :, w0:w0 + 128],
                            in_=e[:, :, w0:w0 + 128],
                            pattern=[[0, 2], [1, 128]],
                            channel_multiplier=-1,
                            base=0,
                            compare_op=is_ge,
                            fill=0.0,
                        )
                    return e, w0

                def av(acc, p, j, m, e, lo):
                    mlast = 4 * j + 3
                    nc.tensor.matmul(
                        out=acc[:, 0, lo:512],
                        lhsT=VA[:, p, m, 0, :, :],
                        rhs=e[:, 0, lo:512],
                        start=(m == 0), stop=(m == mlast),
                    )
                    nc.tensor.matmul(
                        out=acc[:, 1, lo:512],
                        lhsT=VA[:, p, m, 1, :, :],
                        rhs=e[:, 1, lo:512],
                        start=(m == 0), stop=(m == mlast),
                    )

                def normalize(acc, p, j, nsplit=1):
                    # ctx[v, q] = acc[v, q] / acc[64+v, q] for both heads
                    w = 512 // nsplit
                    for h in range(nsplit):
                        lo, hi = w * h, w * (h + 1)
                        sums = work.tile([64, 2, w], F32, tag="sums", bufs=2, name="sums")
                        nc.vector.tensor_copy(out=sums[:], in_=acc[64:128, :, lo:hi])
                        rec = work.tile([64, 2, w], F32, tag="rec", bufs=2, name="rec")
                        nc.vector.reciprocal_approx_fast(rec[:], sums[:])
                        nc.vector.tensor_tensor(
                            out=ctxp[p][0:64, 512 * j + lo:512 * j + hi],
                            in0=acc[0:64, 0, lo:hi],
                            in1=rec[:, 0, :],
                            op=mult,
                        )
                        codd = work.tile([64, w], BF16, tag="codd", bufs=2, name="codd")
                        nc.vector.tensor_tensor(
                            out=codd[:], in0=acc[0:64, 1, lo:hi], in1=rec[:, 1, :],
                            op=mult)
                        nc.vector.tensor_copy(
                            out=ctxp[p][64:128, 512 * j + lo:512 * j + hi], in_=codd[:])

                # software-pipelined main loop: AV(k-1) is emitted after
                # scores(k) so the PE never head-blocks on exp(k-1); filler
                # (proj / outproj) matmuls pace in to keep the PE dense.
                cur_acc = None
                pend = None   # (acc, p, j, m, e, lo)
                for idx, (p, j, m) in enumerate(steps):
                    if m == 0:
                        # new block: fresh accumulator (WAR on previous
                        # block's normalize is absorbed by boundary filler)
                        cur_acc = psum.tile(
                            [128, 2, 512], F32, tag="acc", bufs=1, name="acc")
                        drain_filler(idx, 4)
                        # Q pair of this block and K pair cols up to 512(j+1)
                        # must be fully emitted before its scores
                        require(idx, ("qk", p, j), ("qk", 2 + p, j))
                    e, w0 = scores_exp(p, j, m)
                    import math
                    need = max(0, math.ceil(target_at[idx + 1]) - fill_state["emitted"])
                    budget = max(2, need)
                    drain_filler(idx, max(1, budget // 2), allow_op=False)
                    if pend is not None:
                        pacc, pp, pj, pm, pe, plo = pend
                        require(idx, ("v", pm))
                        av(pacc, pp, pj, pm, pe, plo)
                        if pm == 4 * pj + 3:
                            normalize(pacc, pp, pj)
                    pend = (cur_acc, p, j, m, e, w0)
                    drain_filler(idx, budget - budget // 2)
                pacc, pp, pj, pm, pe, plo = pend
                require(n_steps, ("v", pm))
                av(pacc, pp, pj, pm, pe, plo)
                # split so the tail outproj can start on the first half early
                normalize(pacc, pp, pj, nsplit=2)
                # keep the PE clock warm across the final normalize latency
                # so the tail outproj runs at full rate
                psd2 = psum.tile([128, 128], F32, tag="p", bufs=2, name="psd")
                for k in range(32):
                    nc.tensor.matmul(
                        out=psd2[:], lhsT=dummy[:], rhs=dummy[:],
                        start=(k == 0), stop=(k == 31))
                scr_t = work.tile([1, 16], F32, name="scr_t")
                nc.vector.tensor_copy(out=scr_t[:], in_=psd2[0:1, 0:16])
                nc.sync.dma_start(out=scr_d[:], in_=scr_t[:])
                # drain any remaining filler (final outproj blocks)
                while fill_state["idx"] < len(filler):
                    if drain_filler(n_steps + 1, 1 << 30) == 0:
                        break

    nc.finalize()
    return nc


def kernel(x, qkv_w, qkv_b, out_w, out_b):
    from concourse import bass_utils
    import ml_dtypes
    global last_exec_time_ns

    BF = ml_dtypes.bfloat16

    x = np.asarray(x, dtype=np.float32)
    qkv_w = np.asarray(qkv_w, dtype=np.float32)
    qkv_b = np.asarray(qkv_b, dtype=np.float32)
    out_w = np.asarray(out_w, dtype=np.float32)
    out_b = np.asarray(out_b, dtype=np.float32)

    has_qkvb = bool(np.any(qkv_b))
    if has_qkvb not in _cache:
        _cache[has_qkvb] = _build(has_qkvb)
    nc = _cache[has_qkvb]

    in_maps = []
    for c in range(N_CORES):
        b, hg = divmod(c, HG)
        xT = np.ascontiguousarray(x[b].T.astype(BF))
        rows = np.concatenate([
            qkv_w[DL * hg:DL * (hg + 1)],
            qkv_w[D_MODEL + DL * hg:D_MODEL + DL * (hg + 1)],
            qkv_w[2 * D_MODEL + DL * hg:2 * D_MODEL + DL * (hg + 1)],
        ], axis=0)
        wqkvT = np.ascontiguousarray(rows.T.astype(BF))
        woT = np.ascontiguousarray(out_w[:, DL * hg:DL * (hg + 1)].T.astype(BF))
        m = {"xT": xT, "wqkvT": wqkvT, "woT": woT}
        if has_qkvb:
            m["qb"] = np.concatenate([
                qkv_b[DL * hg:DL * (hg + 1)],
                qkv_b[D_MODEL + DL * hg:D_MODEL + DL * (hg + 1)],
                qkv_b[2 * D_MODEL + DL * hg:2 * D_MODEL + DL * (hg + 1)],
            ]).reshape(1, 3 * DL).astype(BF)
        in_maps.append(m)

    res = bass_utils.run_bass_kernel_spmd(nc, in_maps, core_ids=list(range(N_CORES)))
    last_exec_time_ns = res.exec_time_ns

    out = np.zeros((B, S, D_MODEL), dtype=np.float32)
    for c in range(N_CORES):
        b, hg = divmod(c, HG)
        out[b] += np.asarray(res.results[c]["out"], dtype=np.float32)
    out += out_b[None, None, :]
    return out


# revision 34
# speedup vs baseline: 1.0691x; 1.0139x over previous
import sys

sys.path.insert(0, "/opt/trn_rl_repo")

import numpy as np

D_MODEL = 1024
NUM_HEADS = 16
HEAD_DIM = 64
B = 2
S = 2048
N_CORES = 8
HG = 4          # head-groups (cores per batch)
HPC = 4         # heads per core
DL = 256        # local feature width per core (HPC * HEAD_DIM)

_cache = {}
last_exec_time_ns = None


def _build(has_qkvb):
    import concourse.bacc as bacc
    import concourse.mybir as mybir
    import concourse.tile as tile

    F32 = mybir.dt.float32
    BF16 = mybir.dt.bfloat16
    Exp = mybir.ActivationFunctionType.Exp
    mult = mybir.AluOpType.mult
    is_ge = mybir.AluOpType.is_ge

    nc = bacc.Bacc("TRN2", target_bir_lowering=False, debug=False)
    xT_d = nc.dram_tensor("xT", (D_MODEL, S), BF16, kind="ExternalInput")
    wq_d = nc.dram_tensor("wqkvT", (D_MODEL, 3 * DL), BF16, kind="ExternalInput")
    wo_d = nc.dram_tensor("woT", (DL, D_MODEL), BF16, kind="ExternalInput")
    if has_qkvb:
        qb_d = nc.dram_tensor("qb", (1, 3 * DL), BF16, kind="ExternalInput")
    out_d = nc.dram_tensor("out", (S, D_MODEL), F32, kind="ExternalOutput")
    # tiny scratch sink keeping the HAM-warming chains live
    scr_d = nc.dram_tensor("scr", (1, 64), F32, kind="ExternalOutput")

    with tile.TileContext(nc) as tc:
        with tc.tile_pool(name="persist", bufs=1) as persist:
            xt = [persist.tile([128, S], BF16, name=f"xt{i}") for i in range(8)]
            wq = [persist.tile([128, 3 * DL], BF16, name=f"wq{i}") for i in range(8)]
            # Q/K packed per head-pair p: partitions 0:64 head 2p, 64:128 head 2p+1
            QT = [persist.tile([128, S], BF16, name=f"QT{p}") for p in range(2)]
            KT = [persist.tile([128, S], BF16, name=f"KT{p}") for p in range(2)]
            # V augmented, single tile: [pair, st, head-parity, (v|ones), 64]
            VA = persist.tile([128, 2, 16, 2, 2, 64], BF16, name="VA")
            # ctx pair-packed: head 2p at partitions 0:64, head 2p+1 at 64:128
            ctxp = [persist.tile([128, S], BF16, name=f"ctxp{p}") for p in range(2)]
            wop = [persist.tile([128, D_MODEL], BF16, name=f"wop{p}") for p in range(2)]

            # input DMAs spread across engine queues for issue parallelism
            # first 512 cols of x land first so the n=0 projection group is
            # not serialized behind the full x transfer; spread across the
            # three DMA-capable queues (sync, gpsimd, scalar)
            qeng = [nc.sync, nc.sync, nc.sync, nc.gpsimd,
                    nc.gpsimd, nc.gpsimd, nc.scalar, nc.scalar]
            for i in range(8):
                qeng[i].dma_start(
                    out=wq[i][:, 0:384], in_=wq_d[128 * i:128 * (i + 1), 0:384])
                qeng[i].dma_start(
                    out=xt[i][:, 0:512], in_=xT_d[128 * i:128 * (i + 1), 0:512])
            for i in range(8):
                qeng[i].dma_start(
                    out=wq[i][:, 384:768], in_=wq_d[128 * i:128 * (i + 1), 384:768])
            for p in range(2):
                nc.scalar.dma_start(out=wop[p][:], in_=wo_d[128 * p:128 * (p + 1), :])
            for i in range(8):
                qeng[i].dma_start(
                    out=xt[i][:, 512:1024], in_=xT_d[128 * i:128 * (i + 1), 512:1024])
            for i in range(4):
                nc.sync.dma_start(
                    out=xt[i][:, 1024:2048],
                    in_=xT_d[128 * i:128 * (i + 1), 1024:2048])
            for i in range(4, 8):
                nc.gpsimd.dma_start(
                    out=xt[i][:, 1024:2048],
                    in_=xT_d[128 * i:128 * (i + 1), 1024:2048])

            # (VA ones-memset happens after the HAM-warm dummy memset below)

            with tc.tile_pool(name="work", bufs=1) as work, \
                 tc.tile_pool(name="psum", bufs=1, space="PSUM") as psum:

                if has_qkvb:
                    qb_t = persist.tile([1, 3 * DL], BF16, name="qb_t")
                    nc.sync.dma_start(out=qb_t[:], in_=qb_d[:])
                    ones_t = persist.tile([1, 512], BF16, name="ones_t")
                    nc.vector.memset(ones_t[:], 1.0)

                # HAM pre-warm: dummy matmul chain fills the input-DMA wait
                # so the PE clock is at 2.4GHz when real work starts; its
                # result feeds the exp-table preload (keeps the chain live)
                dummy = work.tile([128, 128], BF16, tag="dummy", bufs=1, name="dummy")
                nc.vector.memset(dummy[:], 0.0)
                psd = psum.tile([128, 128], F32, tag="p", bufs=2, name="psd")
                for k in range(64):
                    nc.tensor.matmul(
                        out=psd[:], lhsT=dummy[:], rhs=dummy[:],
                        start=(k == 0), stop=(k == 63))
                warm = work.tile([1, 16], F32, name="warm")
                nc.vector.tensor_copy(out=warm[:], in_=psd[0:1, 0:16])
                nc.scalar.activation(warm[:], warm[:], Exp, scale=1.0)
                # ones columns of VA (v columns are overwritten by v_proj)
                nc.vector.memset(VA[:], 1.0)

                # ---- filler emitters (each yields per-matmul granularity) ----

                def qk_proj(mi, n):
                    # psq [128 qk-dims, 512 keys]; mi 0,1 = Q pairs; 2,3 = K pairs
                    dst = QT[mi] if mi < 2 else KT[mi - 2]
                    psq = psum.tile([128, 512], F32, tag="p", bufs=2, name="psq")
                    for i in range(8):
                        yield
                        nc.tensor.matmul(
                            out=psq[:],
                            lhsT=wq[i][:, 128 * mi:128 * (mi + 1)],
                            rhs=xt[i][:, 512 * n:512 * (n + 1)],
                            start=(i == 0),
                            stop=(i == 7 and not has_qkvb),
                        )
                    if has_qkvb:
                        nc.tensor.matmul(
                            out=psq[:],
                            lhsT=qb_t[0:1, 128 * mi:128 * (mi + 1)],
                            rhs=ones_t[0:1, :],
                            start=False, stop=True,
                        )
                    nc.vector.tensor_copy(out=dst[:, 512 * n:512 * (n + 1)], in_=psq[:])

                def v_proj(st):
                    # psv [128 keys, (pair, parity, 64)]
                    psv = psum.tile([128, 2, 2, 64], F32, tag="p", bufs=2, name="psv")
                    for i in range(8):
                        yield
                        nc.tensor.matmul(
                            out=psv[:],
                            lhsT=xt[i][:, 128 * st:128 * (st + 1)],
                            rhs=wq[i][:, 512:768],
                            start=(i == 0),
                            stop=(i == 7 and not has_qkvb),
                        )
                    if has_qkvb:
                        nc.tensor.matmul(
                            out=psv[:],
                            lhsT=ones_t[0:1, 0:128],
                            rhs=qb_t[0:1, 512:768],
                            start=False, stop=True,
                        )
                    nc.vector.tensor_copy(out=VA[:, :, st, :, 0, :], in_=psv[:])

                def out_proj(qm):
                    stage = work.tile([128, D_MODEL], F32, tag="st", bufs=2, name="stage")
                    for nh in range(2):
                        pso = psum.tile([128, 512], F32, tag="p", bufs=2, name="pso")
                        for p in range(2):
                            yield
                            nc.tensor.matmul(
                                out=pso[:],
                                lhsT=ctxp[p][:, 128 * qm:128 * (qm + 1)],
                                rhs=wop[p][:, 512 * nh:512 * (nh + 1)],
                                start=(p == 0), stop=(p == 1),
                            )
                        nc.vector.tensor_copy(
                            out=stage[:, 512 * nh:512 * (nh + 1)], in_=pso[:])
                    nc.sync.dma_start(out=out_d[128 * qm:128 * (qm + 1), :], in_=stage[:])

                # global attention step sequence and filler queue with gates.
                # gate = number of attention steps that must be EMITTED before
                # the filler unit may start (keeps FIFO order consistent with
                # data deps, e.g. out_proj needs its ctx block normalized).
                steps = [(p, j, m) for j in range(4) for p in range(2)
                         for m in range(4 * j + 4)]
                n_steps = len(steps)   # 80
                # step index right after block (p,j) finishes:
                end_of = {}
                acc = 0
                for j in range(4):
                    for p in range(2):
                        acc += 4 * j + 4
                        end_of[(p, j)] = acc

                filler = []   # list of (gate, key, generator, deadline, weight)

                def add(gate, key, gen, deadline, weight=8):
                    filler.append((gate, key, gen, deadline, weight))

                def block_start(p, j):
                    return end_of[(p, j)] - (4 * j + 4)

                def v_deadline(st):
                    # AV reading VA[st] is first emitted one step after the
                    # step (0, st//4, m=st)
                    return block_start(0, st // 4) + (st % 4) + 1

                # outproj of j-block may only be emitted AFTER normalize(1,j)
                # has been emitted, which happens while processing the step at
                # index end_of[(1,j)] — so its gate is end_of+1.
                def ham_fill(idx, n):
                    # dummy matmuls bridging DMA-paced ramp gaps (keeps HAM warm)
                    psf = psum.tile([128, 128], F32, tag="p", bufs=2, name="psd")
                    for k in range(n):
                        yield
                        nc.tensor.matmul(
                            out=psf[:], lhsT=dummy[:], rhs=dummy[:],
                            start=(k == 0), stop=(k == n - 1))
                    w2 = work.tile([1, 16], F32, tag="scr", bufs=2, name="w2")
                    nc.vector.tensor_copy(out=w2[:], in_=psf[0:1, 0:16])
                    nc.sync.dma_start(
                        out=scr_d[0:1, 16 * idx + 16:16 * idx + 32], in_=w2[:])

                add(0, ("qk", 0, 0), qk_proj(0, 0), 0)
                add(0, ("hf", 0), ham_fill(0, 16), 1, 16)
                add(0, ("qk", 2, 0), qk_proj(2, 0), 0)
                add(0, ("hf", 1), ham_fill(1, 16), 2, 16)
                for st in range(0, 4):
                    add(0, ("v", st), v_proj(st), v_deadline(st))
                add(0, ("qk", 1, 0), qk_proj(1, 0), block_start(1, 0))
                add(0, ("qk", 3, 0), qk_proj(3, 0), block_start(1, 0))
                add(0, ("qk", 0, 1), qk_proj(0, 1), block_start(0, 1))
                add(0, ("qk", 2, 1), qk_proj(2, 1), block_start(0, 1))
                for st in range(4, 8):
                    add(0, ("v", st), v_proj(st), v_deadline(st))
                add(0, ("qk", 1, 1), qk_proj(1, 1), block_start(1, 1))
                add(0, ("qk", 3, 1), qk_proj(3, 1), block_start(1, 1))
                for qm in range(0, 4):
                    add(end_of[(1, 0)] + 2, ("op", qm), out_proj(qm),
                        end_of[(1, 0)] + 3 + 2 * (qm % 4), 4)
                add(0, ("qk", 0, 2), qk_proj(0, 2), block_start(0, 2))
                add(0, ("qk", 2, 2), qk_proj(2, 2), block_start(0, 2))
                for st in range(8, 12):
                    add(0, ("v", st), v_proj(st), v_deadline(st))
                add(0, ("qk", 1, 2), qk_proj(1, 2), block_start(1, 2))
                add(0, ("qk", 3, 2), qk_proj(3, 2), block_start(1, 2))
                for qm in range(4, 8):
                    add(end_of[(1, 1)] + 2, ("op", qm), out_proj(qm),
                        end_of[(1, 1)] + 3 + 2 * (qm % 4), 4)
                add(0, ("qk", 0, 3), qk_proj(0, 3), block_start(0, 3))
                add(0, ("qk", 2, 3), qk_proj(2, 3), block_start(0, 3))
                for st in range(12, 16):
                    add(0, ("v", st), v_proj(st), v_deadline(st))
                add(0, ("qk", 1, 3), qk_proj(1, 3), block_start(1, 3))
                add(0, ("qk", 3, 3), qk_proj(3, 3), block_start(1, 3))
                for qm in range(8, 12):
                    add(end_of[(1, 2)] + 2, ("op", qm), out_proj(qm),
                        end_of[(1, 2)] + 3 + 2 * (qm % 4), 4)
                for qm in range(12, 16):
                    add(end_of[(1, 3)] + 1, ("op", qm), out_proj(qm),
                        n_steps + 1, 4)

                # piecewise-linear emission target: by the start of step
                # `deadline` the unit must be fully emitted; spread the work
                # evenly over the steps before it
                pts = []
                cum = 0
                dmax = 0
                for gate, key, gen, dl, w in filler:
                    dmax = max(dmax, dl)
                    cum += w
                    pts.append((dmax, cum))
                target_at = [0.0] * (n_steps + 2)
                prev_d, prev_c = 0, 0.0
                for dl, c in pts:
                    if dl > prev_d:
                        for s in range(prev_d, min(dl, n_steps + 2)):
                            target_at[s] = prev_c + (c - prev_c) * (s - prev_d) / (dl - prev_d)
                        prev_d, prev_c = dl, float(c)
                    else:
                        prev_c = float(c)
                for s in range(prev_d, n_steps + 2):
                    target_at[s] = prev_c

                fill_state = {"emitted": 0, "idx": 0}
                produced = set()

                def drain_filler(step_idx, budget, allow_op=True):
                    done = 0
                    while done < budget and fill_state["idx"] < len(filler):
                        gate, key, gen, _dl, _w = filler[fill_state["idx"]]
                        if gate > step_idx:
                            break
                        if not allow_op and key[0] == "op":
                            # outproj blocks on the previous normalize; it may
                            # only enter the PE queue after the pending AV
                            break
                        try:
                            next(gen)
                            done += 1
                            fill_state["emitted"] += 1
                        except StopIteration:
                            produced.add(key)
                            fill_state["idx"] += 1
                    return done

                def require(step_idx, *keys):
                    # force-drain filler (in order, respecting gates) until
                    # the named units have fully emitted
                    while any(k not in produced for k in keys):
                        if drain_filler(step_idx, 1) == 0:
                            raise RuntimeError(f"unsatisfiable requires {keys}")

                def scores_exp(p, j, m):
                    t = m - 4 * j
                    w0 = 128 * t if t > 0 else 0
                    psS = psum.tile([128, 2, 512], F32, tag="s", bufs=2, name="psS")
                    nc.tensor.matmul(
                        out=psS[:, 0, w0:512],
                        lhsT=KT[p][0:64, 128 * m:128 * (m + 1)],
                        rhs=QT[p][0:64, 512 * j + w0:512 * (j + 1)],
                        start=True, stop=True,
                        tile_position=(0, 0),
                    )
                    nc.tensor.matmul(
                        out=psS[:, 1, w0:512],
                        lhsT=KT[p][64:128, 128 * m:128 * (m + 1)],
                        rhs=QT[p][64:128, 512 * j + w0:512 * (j + 1)],
                        start=True, stop=True,
                        tile_position=(64, 0),
                    )
                    e = work.tile([128, 2, 512], BF16, tag="e", bufs=4, name="e")
                    nc.scalar.activation(
                        e[:, :, w0:512], psS[:, :, w0:512], Exp, scale=0.125)
                    if t >= 0:
                        # causal band: keep where col - key >= 0 (both heads)
                        nc.gpsimd.affine_select(
                            out=e[:, :, w0:w0 + 128],
                            in_=e[:, :, w0:w0 + 128],
                            pattern=[[0, 2], [1, 128]],
                            channel_multiplier=-1,
                            base=0,
                            compare_op=is_ge,
                            fill=0.0,
                        )
                    return e, w0

                def av(acc, p, j, m, e, lo):
                    mlast = 4 * j + 3
                    nc.tensor.matmul(
                        out=acc[:, 0, lo:512],
                        lhsT=VA[:, p, m, 0, :, :],
                        rhs=e[:, 0, lo:512],
                        start=(m == 0), stop=(m == mlast),
                    )
                    nc.tensor.matmul(
                        out=acc[:, 1, lo:512],
                        lhsT=VA[:, p, m, 1, :, :],
                        rhs=e[:, 1, lo:512],
                        start=(m == 0), stop=(m == mlast),
                    )

                def normalize(acc, p, j, nsplit=1):
                    # ctx[v, q] = acc[v, q] / acc[64+v, q] for both heads
                    w = 512 // nsplit
                    for h in range(nsplit):
                        lo, hi = w * h, w * (h + 1)
                        sums = work.tile([64, 2, w], F32, tag="sums", bufs=2, name="sums")
                        nc.vector.tensor_copy(out=sums[:], in_=acc[64:128, :, lo:hi])
                        rec = work.tile([64, 2, w], F32, tag="rec", bufs=2, name="rec")
                        nc.vector.reciprocal_approx_fast(rec[:], sums[:])
                        nc.vector.tensor_tensor(
                            out=ctxp[p][0:64, 512 * j + lo:512 * j + hi],
                            in0=acc[0:64, 0, lo:hi],
                            in1=rec[:, 0, :],
                            op=mult,
                        )
                        codd = work.tile([64, w], BF16, tag="codd", bufs=2, name="codd")
                        nc.vector.tensor_tensor(
                            out=codd[:], in0=acc[0:64, 1, lo:hi], in1=rec[:, 1, :],
                            op=mult)
                        nc.vector.tensor_copy(
                            out=ctxp[p][64:128, 512 * j + lo:512 * j + hi], in_=codd[:])

                # software-pipelined main loop: AV(k-1) is emitted after
                # scores(k) so the PE never head-blocks on exp(k-1); filler
                # (proj / outproj) matmuls pace in to keep the PE dense.
                cur_acc = None
                pend = None   # (acc, p, j, m, e, lo)
                for idx, (p, j, m) in enumerate(steps):
                    if m == 0:
                        # new block: fresh accumulator (WAR on previous
                        # block's normalize is absorbed by boundary filler)
                        cur_acc = psum.tile(
                            [128, 2, 512], F32, tag="acc", bufs=1, name="acc")
                        drain_filler(idx, 4)
                        # Q pair of this block and K pair cols up to 512(j+1)
                        # must be fully emitted before its scores
                        require(idx, ("qk", p, j), ("qk", 2 + p, j))
                    e, w0 = scores_exp(p, j, m)
                    import math
                    need = max(0, math.ceil(target_at[idx + 1]) - fill_state["emitted"])
                    budget = max(2, need)
                    drain_filler(idx, max(1, budget // 2), allow_op=False)
                    if pend is not None:
                        pacc, pp, pj, pm, pe, plo = pend
                        require(idx, ("v", pm))
                        av(pacc, pp, pj, pm, pe, plo)
                        if pm == 4 * pj + 3:
                            normalize(pacc, pp, pj)
                    pend = (cur_acc, p, j, m, e, w0)
                    drain_filler(idx, budget - budget // 2)
                pacc, pp, pj, pm, pe, plo = pend
                require(n_steps, ("v", pm))
                av(pacc, pp, pj, pm, pe, plo)
                # split so the tail outproj can start on the first half early
                normalize(pacc, pp, pj, nsplit=2)
                # keep the PE clock warm across the final normalize latency
                # so the tail outproj runs at full rate
                psd2 = psum.tile([128, 128], F32, tag="p", bufs=2, name="psd")
                for k in range(64):
                    nc.tensor.matmul(
                        out=psd2[:], lhsT=dummy[:], rhs=dummy[:],
                        start=(k == 0), stop=(k == 63))
                scr_t = work.tile([1, 16], F32, name="scr_t")
                nc.vector.tensor_copy(out=scr_t[:], in_=psd2[0:1, 0:16])
                nc.sync.dma_start(out=scr_d[0:1, 0:16], in_=scr_t[:])
                # drain any remaining filler (final outproj blocks)
                while fill_state["idx"] < len(filler):
                    if drain_filler(n_steps + 1, 1 << 30) == 0:
                        break

    nc.finalize()
    return nc


def kernel(x, qkv_w, qkv_b, out_w, out_b):
    from concourse import bass_utils
    import ml_dtypes
    global last_exec_time_ns

    BF = ml_dtypes.bfloat16

    x = np.asarray(x, dtype=np.float32)
    qkv_w = np.asarray(qkv_w, dtype=np.float32)
    qkv_b = np.asarray(qkv_b, dtype=np.float32)
    out_w = np.asarray(out_w, dtype=np.float32)
    out_b = np.asarray(out_b, dtype=np.float32)

    has_qkvb = bool(np.any(qkv_b))
    if has_qkvb not in _cache:
        _cache[has_qkvb] = _build(has_qkvb)
    nc = _cache[has_qkvb]

    in_maps = []
    for c in range(N_CORES):
        b, hg = divmod(c, HG)
        xT = np.ascontiguousarray(x[b].T.astype(BF))
        rows = np.concatenate([
            qkv_w[DL * hg:DL * (hg + 1)],
            qkv_w[D_MODEL + DL * hg:D_MODEL + DL * (hg + 1)],
            qkv_w[2 * D_MODEL + DL * hg:2 * D_MODEL + DL * (hg + 1)],
        ], axis=0)
        wqkvT = np.ascontiguousarray(rows.T.astype(BF))
        woT = np.ascontiguousarray(out_w[:, DL * hg:DL * (hg + 1)].T.astype(BF))
        m = {"xT": xT, "wqkvT": wqkvT, "woT": woT}
        if has_qkvb:
            m["qb"] = np.concatenate([
                qkv_b[DL * hg:DL * (hg + 1)],
                qkv_b[D_MODEL + DL * hg:D_MODEL + DL * (hg + 1)],
                qkv_b[2 * D_MODEL + DL * hg:2 * D_MODEL + DL * (hg + 1)],
            ]).reshape(1, 3 * DL).astype(BF)
        in_maps.append(m)

    res = bass_utils.run_bass_kernel_spmd(nc, in_maps, core_ids=list(range(N_CORES)))
    last_exec_time_ns = res.exec_time_ns

    out = np.zeros((B, S, D_MODEL), dtype=np.float32)
    for c in range(N_CORES):
        b, hg = divmod(c, HG)
        out[b] += np.asarray(res.results[c]["out"], dtype=np.float32)
    out += out_b[None, None, :]
    return out
